# revision 5
# baseline (speedup 1.0000x reference)
"""Trainium2 Bass kernel for nn_CrossTransformer (cross-attention over support set).

Contract: kernel(**inputs) takes FULL inputs (query_repr [8,512,20,20],
supports_repr [8,5,5,512,20,20], Wqk [128,512], Wv [128,512]) and returns the
full outputs (query_v [8,128,20,20], out [8,5,128,20,20]) as float32, matching
reference():
    query_v = Wv @ query_repr                       (1x1 conv)
    sim     = (Wqk @ query)^T (Wqk @ supports) * 128^-0.5
    attn    = softmax over (n,i,j) of supports
    out     = attn @ (Wv @ supports)

Sharding: pure data-parallel over batch b -- each of the 8 NeuronCores handles
one batch element; no collectives.

Per-core layout strategy (everything stays transposed so no on-chip transposes
are ever needed):
  - sim^T [nij, hw] = ks^T q with lhsT = ks (natural), rhs = q (natural)
  - v^T [nij, c] produced directly by using s-chunks as the matmul weights
  - softmax over nij (partition dim): denominator via DVE accumulation of the
    exp chunks + a single ones-vector matmul; normalization by an outer-product
    broadcast matmul. exp() needs no max-subtraction: sim*scale ~ N(0,1),
    |sim*scale| < ~7 for this distribution, comfortably inside f32 exp range.
"""

import numpy as np
from contextlib import ExitStack

import concourse.bass as bass
import concourse.tile as tile
from concourse import bacc, mybir
from concourse.bass_utils import run_bass_kernel_spmd

# Problem shape (hardcoded per harness contract)
B, K, N, C, D = 8, 5, 5, 512, 128
H = W = 20
HW = H * W                 # 400
NIJ = N * HW               # 2000
CC = C // 128              # 4 contraction chunks of 128
NT = 16                    # nij chunks
CH = NIJ // NT             # 125 rows per chunk
KSJ = 4                    # ks free-dim chunks
KSW = NIJ // KSJ           # 500 (<= 512 f32 moving-operand limit)
SCALE = float(D) ** -0.5
F32 = mybir.dt.float32

_CACHE = {}


def _build_program():
    nc = bacc.Bacc("TRN2", target_bir_lowering=False, debug=False)

    xq_d = nc.dram_tensor("xq", [CC, 128, HW], F32, kind="ExternalInput").ap()
    s_d = nc.dram_tensor("s", [K, CC, 128, NIJ], F32, kind="ExternalInput").ap()
    wq_d = nc.dram_tensor("wqkT", [CC, 128, D], F32, kind="ExternalInput").ap()
    wv_d = nc.dram_tensor("wvT", [CC, 128, D], F32, kind="ExternalInput").ap()
    qv_d = nc.dram_tensor("qv", [D, HW], F32, kind="ExternalOutput").ap()
    out_d = nc.dram_tensor("out", [K, D, HW], F32, kind="ExternalOutput").ap()

    with ExitStack() as ctx:
        tc = ctx.enter_context(tile.TileContext(nc))

        consts = ctx.enter_context(tc.tile_pool(name="consts", bufs=1))
        spool = ctx.enter_context(tc.tile_pool(name="spool", bufs=2))
        kvpool = ctx.enter_context(tc.tile_pool(name="kvpool", bufs=2))
        epool = ctx.enter_context(tc.tile_pool(name="epool", bufs=4))
        opool = ctx.enter_context(tc.tile_pool(name="opool", bufs=2))
        ps_ks = ctx.enter_context(tc.tile_pool(name="ps_ks", bufs=2, space="PSUM"))
        ps_vt = ctx.enter_context(tc.tile_pool(name="ps_vt", bufs=2, space="PSUM"))
        ps_sim = ctx.enter_context(tc.tile_pool(name="ps_sim", bufs=2, space="PSUM"))
        ps_out = ctx.enter_context(tc.tile_pool(name="ps_out", bufs=2, space="PSUM"))

        # ---- constants / per-batch tensors ----
        wq_sb = consts.tile([128, CC, D], F32)
        wv_sb = consts.tile([128, CC, D], F32)
        xq_sb = consts.tile([128, CC, HW], F32)
        for cc in range(CC):
            nc.sync.dma_start(out=wq_sb[:, cc, :], in_=wq_d[cc])
            nc.sync.dma_start(out=wv_sb[:, cc, :], in_=wv_d[cc])
            nc.sync.dma_start(out=xq_sb[:, cc, :], in_=xq_d[cc])
        ones_col = consts.tile([CH, 1], F32)
        nc.vector.memset(ones_col, 1.0)
        ones_row = consts.tile([1, D], F32)
        nc.vector.memset(ones_row, 1.0)

        # ---- q / qv projections (once per batch) ----
        q_ps = ps_out.tile([D, HW], F32, tag="out_ps")
        for cc in range(CC):
            nc.tensor.matmul(q_ps, wq_sb[:, cc, :], xq_sb[:, cc, :],
                             start=(cc == 0), stop=(cc == CC - 1))
        q_sb = consts.tile([D, HW], F32)
        nc.vector.tensor_copy(q_sb, q_ps)

        qv_ps = ps_out.tile([D, HW], F32, tag="out_ps")
        for cc in range(CC):
            nc.tensor.matmul(qv_ps, wv_sb[:, cc, :], xq_sb[:, cc, :],
                             start=(cc == 0), stop=(cc == CC - 1))
        qv_sb = consts.tile([D, HW], F32)
        nc.vector.tensor_copy(qv_sb, qv_ps)
        nc.sync.dma_start(out=qv_d, in_=qv_sb)

        # ---- per class-slot k ----
        for k in range(K):
            # load supports slice: 4 x [128, 2000] (1 MB contiguous DMAs)
            s_sb = [spool.tile([128, NIJ], F32, tag=f"s{cc}", name=f"s_sb{cc}")
                    for cc in range(CC)]
            for cc in range(CC):
                nc.sync.dma_start(out=s_sb[cc], in_=s_d[k, cc])

            # ks[d, nij] = Wqk @ s   (weight-stationary)
            ks_sb = kvpool.tile([128, NIJ], F32, tag="ks")
            for j in range(KSJ):
                ks_ps = ps_ks.tile([D, KSW], F32, tag="ks_ps")
                for cc in range(CC):
                    nc.tensor.matmul(ks_ps, wq_sb[:, cc, :],
                                     s_sb[cc][:, j * KSW:(j + 1) * KSW],
                                     start=(cc == 0), stop=(cc == CC - 1))
                nc.vector.tensor_copy(ks_sb[:, j * KSW:(j + 1) * KSW], ks_ps)

            # vT[nij, c] = (s^T) @ WvT  (s-chunks as weights -> transposed out)
            vt_sb = kvpool.tile([CH, NT, D], F32, tag="vt")
            for t in range(NT):
                vt_ps = ps_vt.tile([CH, D], F32, tag="vt_ps")
                for cc in range(CC):
                    nc.tensor.matmul(vt_ps, s_sb[cc][:, t * CH:(t + 1) * CH],
                                     wv_sb[:, cc, :],
                                     start=(cc == 0), stop=(cc == CC - 1))
                nc.vector.tensor_copy(vt_sb[:, t, :], vt_ps)

            # attention chunks: simT -> exp -> accumulate out and expsum
            o_ps = ps_out.tile([D, HW], F32, tag="out_ps")
            eacc = epool.tile([CH, HW], F32, tag="eacc")
            for t in range(NT):
                sim_ps = ps_sim.tile([CH, HW], F32, tag="sim_ps")
                nc.tensor.matmul(sim_ps, ks_sb[:, t * CH:(t + 1) * CH], q_sb,
                                 start=True, stop=True)
                e_sb = epool.tile([CH, HW], F32, tag="e")
                nc.scalar.activation(e_sb, sim_ps,
                                     mybir.ActivationFunctionType.Exp,
                                     scale=SCALE)
                nc.tensor.matmul(o_ps, vt_sb[:, t, :], e_sb,
                                 start=(t == 0), stop=(t == NT - 1))
                if t == 0:
                    nc.vector.tensor_copy(eacc, e_sb)
                else:
                    nc.vector.tensor_add(eacc, eacc, e_sb)

            # softmax denominator: ones-matmul partition reduction -> 1/x
            sum_ps = ps_sim.tile([1, HW], F32, tag="sim_ps")
            nc.tensor.matmul(sum_ps, ones_col, eacc, start=True, stop=True)
            recip = epool.tile([1, HW], F32, tag="recip")
            nc.vector.reciprocal(recip, sum_ps)
            # broadcast across partitions via outer product, then normalize
            bc_ps = ps_sim.tile([D, HW], F32, tag="sim_ps")
            nc.tensor.matmul(bc_ps, ones_row, recip, start=True, stop=True)
            bc_sb = epool.tile([D, HW], F32, tag="bc")
            nc.vector.tensor_copy(bc_sb, bc_ps)
            o_sb = opool.tile([D, HW], F32, tag="osb")
            nc.vector.tensor_mul(o_sb, o_ps, bc_sb)
            nc.sync.dma_start(out=out_d[k], in_=o_sb)

    nc.compile()
    return nc


def _get_nc():
    if "nc" not in _CACHE:
        _CACHE["nc"] = _build_program()
    return _CACHE["nc"]


def _prep_core_inputs(query_repr, supports_repr, wqkT, wvT, b):
    xq = np.ascontiguousarray(query_repr[b].reshape(CC, 128, HW))
    # supports [K,N,C,H,W] -> [K, CC, 128, N*HW] with C chunked on partitions
    s = supports_repr[b].reshape(K, N, CC, 128, HW)
    s = np.ascontiguousarray(s.transpose(0, 2, 3, 1, 4)).reshape(K, CC, 128, NIJ)
    return {"xq": xq, "s": s, "wqkT": wqkT, "wvT": wvT}


def kernel(query_repr, supports_repr, Wqk, Wv):
    query_repr = np.asarray(query_repr, dtype=np.float32)
    supports_repr = np.asarray(supports_repr, dtype=np.float32)
    wqkT = np.ascontiguousarray(np.asarray(Wqk, np.float32).T).reshape(CC, 128, D)
    wvT = np.ascontiguousarray(np.asarray(Wv, np.float32).T).reshape(CC, 128, D)

    nc = _get_nc()
    in_maps = [_prep_core_inputs(query_repr, supports_repr, wqkT, wvT, b)
               for b in range(B)]
    res = run_bass_kernel_spmd(nc, in_maps, list(range(B))).results

    query_v = np.stack([res[b]["qv"] for b in range(B)]).reshape(B, D, H, W)
    out = np.stack([res[b]["out"] for b in range(B)]).reshape(B, K, D, H, W)
    return query_v.astype(np.float32), out.astype(np.float32)


# revision 9
# speedup vs baseline: 1.5196x; 1.5196x over previous
"""Trainium2 Bass kernel for nn_CrossTransformer (cross-attention over support set).

Contract: kernel(**inputs) takes FULL inputs (query_repr [8,512,20,20],
supports_repr [8,5,5,512,20,20], Wqk [128,512], Wv [128,512]) and returns the
full outputs (query_v [8,128,20,20], out [8,5,128,20,20]) as float32, matching
reference():
    query_v = Wv @ query_repr                       (1x1 conv)
    sim     = (Wqk @ query)^T (Wqk @ supports) * 128^-0.5
    attn    = softmax over (n,i,j) of supports
    out     = attn @ (Wv @ supports)

Sharding: pure data-parallel over batch b -- each of the 8 NeuronCores handles
one batch element; no collectives.

Per-core strategy (everything stays transposed; no on-chip transposes needed):
  - matmuls run in float32r (single-pass fp32, 4x faster than fp32's LOW_HIGH
    two-pass mode; measured scale-rel err ~1.6e-4 vs 2.3e-3 for bf16). f32r
    needs moving-dim >= 256 for full rate, so the vT projection's rhs is
    padded from 128 to 256 columns (the extra output columns are never read).
  - sim^T [nij, hw] = ks^T q with lhsT = ks (natural), rhs = q (natural)
  - v^T [nij, c] produced directly by using s-chunks as the matmul weights
  - softmax over nij (the partition dim): exp on ScalarE (no max-subtraction
    needed: sim*scale ~ N(0,1), |max| ~ 7, far inside f32 exp range);
    denominator via GpSimd accumulation of exp chunks (keeps DVE free) + one
    ones-vector matmul for the partition reduction; normalization by an
    outer-product broadcast matmul + DVE multiply.
"""

import numpy as np
from contextlib import ExitStack

import concourse.bass as bass
import concourse.tile as tile
from concourse import bacc, mybir
from concourse.bass_utils import run_bass_kernel_spmd

# Problem shape (hardcoded per harness contract)
B, K, N, C, D = 8, 5, 5, 512, 128
H = W = 20
HW = H * W                 # 400
NIJ = N * HW               # 2000
CC = C // 128              # 4 contraction chunks of 128
NT = 16                    # nij chunks
CH = NIJ // NT             # 125 rows per chunk
KSJ = 4                    # ks free-dim chunks
KSW = NIJ // KSJ           # 500 (<= 512 f32 moving-operand limit)
DW = 256                   # vT rhs padded width (f32r full rate needs >=256)
SCALE = float(D) ** -0.5
F32 = mybir.dt.float32
F32R = mybir.dt.float32r

_CACHE = {}


def _r(ap):
    """View an f32 AP as float32r for single-pass PE matmul."""
    return ap.bitcast(F32R)


def _build_program():
    nc = bacc.Bacc("TRN2", target_bir_lowering=False, debug=False)

    xq_d = nc.dram_tensor("xq", [CC, 128, HW], F32, kind="ExternalInput").ap()
    s_d = nc.dram_tensor("s", [K, CC, 128, NIJ], F32, kind="ExternalInput").ap()
    wq_d = nc.dram_tensor("wqkT", [CC, 128, D], F32, kind="ExternalInput").ap()
    wv_d = nc.dram_tensor("wvT", [CC, 128, D], F32, kind="ExternalInput").ap()
    qv_d = nc.dram_tensor("qv", [D, HW], F32, kind="ExternalOutput").ap()
    out_d = nc.dram_tensor("out", [K, D, HW], F32, kind="ExternalOutput").ap()

    with ExitStack() as ctx:
        tc = ctx.enter_context(tile.TileContext(nc))

        consts = ctx.enter_context(tc.tile_pool(name="consts", bufs=1))
        spool = ctx.enter_context(tc.tile_pool(name="spool", bufs=2))
        kvpool = ctx.enter_context(tc.tile_pool(name="kvpool", bufs=2))
        epool = ctx.enter_context(tc.tile_pool(name="epool", bufs=4))
        opool = ctx.enter_context(tc.tile_pool(name="opool", bufs=2))
        # PSUM budget (8 banks): mm_ps shared ks/vt rotation 3 + sim 3 + out 2
        ps_mm = ctx.enter_context(tc.tile_pool(name="ps_mm", bufs=3, space="PSUM"))
        ps_sim = ctx.enter_context(tc.tile_pool(name="ps_sim", bufs=3, space="PSUM"))
        ps_out = ctx.enter_context(tc.tile_pool(name="ps_out", bufs=2, space="PSUM"))

        # ---- constants / per-batch tensors ----
        wq_sb = consts.tile([128, CC, D], F32)
        wv_sb = consts.tile([128, CC, DW], F32)   # [:, :, :D] real, rest pad
        xq_sb = consts.tile([128, CC, HW], F32)
        for cc in range(CC):
            nc.sync.dma_start(out=_r(wq_sb[:, cc, :]), in_=_r(wq_d[cc]))
            nc.sync.dma_start(out=_r(wv_sb[:, cc, :D]), in_=_r(wv_d[cc]))
            # pad cols [D:DW] with a second WvT copy: never read from the
            # vT psum, but f32r matmuls need all-f32r-typed producers
            nc.sync.dma_start(out=_r(wv_sb[:, cc, D:]), in_=_r(wv_d[cc]))
            nc.sync.dma_start(out=_r(xq_sb[:, cc, :]), in_=_r(xq_d[cc]))
        ones_col = consts.tile([CH, 1], F32)
        nc.vector.memset(ones_col, 1.0)
        ones_row = consts.tile([1, D], F32)
        nc.vector.memset(ones_row, 1.0)

        # ---- q / qv projections (once per batch) ----
        q_ps = ps_out.tile([D, HW], F32, tag="out_ps")
        for cc in range(CC):
            nc.tensor.matmul(q_ps, _r(wq_sb[:, cc, :]), _r(xq_sb[:, cc, :]),
                             start=(cc == 0), stop=(cc == CC - 1))
        q_sb = consts.tile([D, HW], F32)
        nc.vector.tensor_copy(_r(q_sb), q_ps)

        qv_ps = ps_out.tile([D, HW], F32, tag="out_ps")
        for cc in range(CC):
            nc.tensor.matmul(qv_ps, _r(wv_sb[:, cc, :D]), _r(xq_sb[:, cc, :]),
                             start=(cc == 0), stop=(cc == CC - 1))
        qv_sb = consts.tile([D, HW], F32)
        nc.vector.tensor_copy(qv_sb, qv_ps)
        nc.sync.dma_start(out=qv_d, in_=qv_sb)

        # ---- per class-slot k ----
        for k in range(K):
            # load supports slice: 4 x [128, 2000] (1 MB contiguous DMAs)
            s_sb = [spool.tile([128, NIJ], F32, tag=f"s{cc}", name=f"s_sb{cc}")
                    for cc in range(CC)]
            for cc in range(CC):
                nc.sync.dma_start(out=_r(s_sb[cc]), in_=_r(s_d[k, cc]))

            # ks[d, nij] = Wqk @ s   (weight-stationary)
            ks_sb = kvpool.tile([128, NIJ], F32, tag="ks")
            for j in range(KSJ):
                ks_ps = ps_mm.tile([D, KSW], F32, tag="mm_ps")
                for cc in range(CC):
                    nc.tensor.matmul(ks_ps, _r(wq_sb[:, cc, :]),
                                     _r(s_sb[cc][:, j * KSW:(j + 1) * KSW]),
                                     start=(cc == 0), stop=(cc == CC - 1))
                nc.vector.tensor_copy(_r(ks_sb[:, j * KSW:(j + 1) * KSW]), ks_ps)

            # vT[nij, c] = (s^T) @ WvT  (s-chunks as weights -> transposed out)
            vt_sb = kvpool.tile([CH, NT, D], F32, tag="vt")
            for t in range(NT):
                vt_ps = ps_mm.tile([CH, DW], F32, tag="mm_ps")
                for cc in range(CC):
                    nc.tensor.matmul(vt_ps, _r(s_sb[cc][:, t * CH:(t + 1) * CH]),
                                     _r(wv_sb[:, cc, :]),
                                     start=(cc == 0), stop=(cc == CC - 1))
                nc.vector.tensor_copy(_r(vt_sb[:, t, :]), vt_ps[:, :D])

            # attention chunks: simT -> exp -> accumulate out and expsum
            o_ps = ps_out.tile([D, HW], F32, tag="out_ps")
            eacc = epool.tile([CH, HW], F32, tag="eacc")
            for t in range(NT):
                sim_ps = ps_sim.tile([CH, HW], F32, tag="sim_ps")
                nc.tensor.matmul(sim_ps, _r(ks_sb[:, t * CH:(t + 1) * CH]),
                                 _r(q_sb), start=True, stop=True)
                e_sb = epool.tile([CH, HW], F32, tag="e")
                nc.scalar.activation(_r(e_sb), sim_ps,
                                     mybir.ActivationFunctionType.Exp,
                                     scale=SCALE)
                nc.tensor.matmul(o_ps, _r(vt_sb[:, t, :]), _r(e_sb),
                                 start=(t == 0), stop=(t == NT - 1))
                if t == 0:
                    nc.gpsimd.tensor_copy(eacc, e_sb)
                else:
                    nc.gpsimd.tensor_add(eacc, eacc, e_sb)

            # softmax denominator: ones-matmul partition reduction -> 1/x
            sum_ps = ps_sim.tile([1, HW], F32, tag="sim_ps")
            nc.tensor.matmul(sum_ps, ones_col, eacc, start=True, stop=True)
            recip = epool.tile([1, HW], F32, tag="recip")
            nc.vector.reciprocal(recip, sum_ps)
            # broadcast across partitions via outer product, then normalize
            bc_ps = ps_sim.tile([D, HW], F32, tag="sim_ps")
            nc.tensor.matmul(bc_ps, ones_row, recip, start=True, stop=True)
            bc_sb = epool.tile([D, HW], F32, tag="bc")
            nc.vector.tensor_copy(bc_sb, bc_ps)
            o_sb = opool.tile([D, HW], F32, tag="osb")
            nc.vector.tensor_mul(o_sb, o_ps, bc_sb)
            nc.sync.dma_start(out=out_d[k], in_=o_sb)

    nc.compile()
    return nc


def _get_nc():
    if "nc" not in _CACHE:
        _CACHE["nc"] = _build_program()
    return _CACHE["nc"]


def _prep_core_inputs(query_repr, supports_repr, wqkT, wvT, b):
    xq = np.ascontiguousarray(query_repr[b].reshape(CC, 128, HW))
    # supports [K,N,C,H,W] -> [K, CC, 128, N*HW] with C chunked on partitions
    s = supports_repr[b].reshape(K, N, CC, 128, HW)
    s = np.ascontiguousarray(s.transpose(0, 2, 3, 1, 4)).reshape(K, CC, 128, NIJ)
    return {"xq": xq, "s": s, "wqkT": wqkT, "wvT": wvT}


def kernel(query_repr, supports_repr, Wqk, Wv):
    query_repr = np.asarray(query_repr, dtype=np.float32)
    supports_repr = np.asarray(supports_repr, dtype=np.float32)
    wqkT = np.ascontiguousarray(np.asarray(Wqk, np.float32).T).reshape(CC, 128, D)
    wvT = np.ascontiguousarray(np.asarray(Wv, np.float32).T).reshape(CC, 128, D)

    nc = _get_nc()
    in_maps = [_prep_core_inputs(query_repr, supports_repr, wqkT, wvT, b)
               for b in range(B)]
    res = run_bass_kernel_spmd(nc, in_maps, list(range(B))).results

    query_v = np.stack([res[b]["qv"] for b in range(B)]).reshape(B, D, H, W)
    out = np.stack([res[b]["out"] for b in range(B)]).reshape(B, K, D, H, W)
    return query_v.astype(np.float32), out.astype(np.float32)


# revision 10
# speedup vs baseline: 1.8886x; 1.2428x over previous
"""Trainium2 Bass kernel for nn_CrossTransformer (cross-attention over support set).

Contract: kernel(**inputs) takes FULL inputs (query_repr [8,512,20,20],
supports_repr [8,5,5,512,20,20], Wqk [128,512], Wv [128,512]) and returns the
full outputs (query_v [8,128,20,20], out [8,5,128,20,20]) as float32, matching
reference():
    query_v = Wv @ query_repr                       (1x1 conv)
    sim     = (Wqk @ query)^T (Wqk @ supports) * 128^-0.5
    attn    = softmax over (n,i,j) of supports
    out     = attn @ (Wv @ supports)

Sharding: pure data-parallel over batch b -- each of the 8 NeuronCores handles
one batch element; no collectives.

Per-core strategy (everything stays transposed; no on-chip transposes needed):
  - matmuls run in float32r (single-pass fp32, 4x faster than fp32's LOW_HIGH
    two-pass mode; measured scale-rel err ~1.6e-4 vs 2.3e-3 for bf16). f32r
    needs moving-dim >= 256 for full rate, so the vT projection's rhs is
    padded from 128 to 256 columns (the extra output columns are never read).
  - sim^T [nij, hw] = ks^T q with lhsT = ks (natural), rhs = q (natural)
  - v^T [nij, c] produced directly by using s-chunks as the matmul weights
  - softmax over nij (the partition dim): exp on ScalarE (no max-subtraction
    needed: sim*scale ~ N(0,1), |max| ~ 7, far inside f32 exp range);
    denominator via GpSimd accumulation of exp chunks (keeps DVE free) + one
    ones-vector matmul for the partition reduction; normalization by an
    outer-product broadcast matmul + DVE multiply.
"""

import numpy as np
from contextlib import ExitStack

import concourse.bass as bass
import concourse.tile as tile
from concourse import bacc, mybir
from concourse.bass_utils import run_bass_kernel_spmd

# Problem shape (hardcoded per harness contract)
B, K, N, C, D = 8, 5, 5, 512, 128
H = W = 20
HW = H * W                 # 400
NIJ = N * HW               # 2000
CC = C // 128              # 4 contraction chunks of 128
NT = 16                    # nij chunks
CH = NIJ // NT             # 125 rows per chunk
KSJ = 4                    # ks free-dim chunks
KSW = NIJ // KSJ           # 500 (<= 512 f32 moving-operand limit)
DW = 256                   # vT rhs padded width (f32r full rate needs >=256)
SCALE = float(D) ** -0.5
F32 = mybir.dt.float32
F32R = mybir.dt.float32r

_CACHE = {}


def _r(ap):
    """View an f32 AP as float32r for single-pass PE matmul."""
    return ap.bitcast(F32R)


def _build_program():
    nc = bacc.Bacc("TRN2", target_bir_lowering=False, debug=False)

    xq_d = nc.dram_tensor("xq", [CC, 128, HW], F32, kind="ExternalInput").ap()
    s_d = nc.dram_tensor("s", [K, CC, 128, NIJ], F32, kind="ExternalInput").ap()
    wq_d = nc.dram_tensor("wqkT", [CC, 128, D], F32, kind="ExternalInput").ap()
    wv_d = nc.dram_tensor("wvT", [CC, 128, D], F32, kind="ExternalInput").ap()
    qv_d = nc.dram_tensor("qv", [D, HW], F32, kind="ExternalOutput").ap()
    out_d = nc.dram_tensor("out", [K, D, HW], F32, kind="ExternalOutput").ap()

    with ExitStack() as ctx:
        tc = ctx.enter_context(tile.TileContext(nc))

        consts = ctx.enter_context(tc.tile_pool(name="consts", bufs=1))
        spool = ctx.enter_context(tc.tile_pool(name="spool", bufs=3))
        kvpool = ctx.enter_context(tc.tile_pool(name="kvpool", bufs=2))
        epool = ctx.enter_context(tc.tile_pool(name="epool", bufs=4))
        opool = ctx.enter_context(tc.tile_pool(name="opool", bufs=2))
        # PSUM budget (8 banks): mm_ps shared ks/vt rotation 3 + sim 3 + out 2
        ps_mm = ctx.enter_context(tc.tile_pool(name="ps_mm", bufs=3, space="PSUM"))
        ps_sim = ctx.enter_context(tc.tile_pool(name="ps_sim", bufs=2, space="PSUM"))
        ps_sum = ctx.enter_context(tc.tile_pool(name="ps_sum", bufs=1, space="PSUM"))
        ps_out = ctx.enter_context(tc.tile_pool(name="ps_out", bufs=2, space="PSUM"))

        # ---- prefetch first supports slice before anything else ----
        s_tiles = {}
        for k in range(min(K, 1)):
            s_tiles[k] = [spool.tile([128, NIJ], F32, tag=f"s{cc}",
                                     name=f"s_sb{k}_{cc}") for cc in range(CC)]
            for cc in range(CC):
                nc.sync.dma_start(out=_r(s_tiles[k][cc]), in_=_r(s_d[k, cc]))

        # ---- constants / per-batch tensors ----
        wq_sb = consts.tile([128, CC, D], F32)
        wv_sb = consts.tile([128, CC, DW], F32)   # [:, :, :D] real, rest pad
        xq_sb = consts.tile([128, CC, HW], F32)
        for cc in range(CC):
            nc.sync.dma_start(out=_r(wq_sb[:, cc, :]), in_=_r(wq_d[cc]))
            nc.sync.dma_start(out=_r(wv_sb[:, cc, :D]), in_=_r(wv_d[cc]))
            # pad cols [D:DW] with a second WvT copy: never read from the
            # vT psum, but f32r matmuls need all-f32r-typed producers
            nc.sync.dma_start(out=_r(wv_sb[:, cc, D:]), in_=_r(wv_d[cc]))
            nc.sync.dma_start(out=_r(xq_sb[:, cc, :]), in_=_r(xq_d[cc]))
        ones_tmp = consts.tile([CH, 1], F32)
        nc.vector.memset(ones_tmp, 1.0)
        ones_col = consts.tile([CH, 1], F32)
        nc.vector.tensor_copy(_r(ones_col), ones_tmp)
        ones_row = consts.tile([1, D], F32)
        nc.vector.memset(ones_row, 1.0)

        # ---- q / qv projections (once per batch) ----
        q_ps = ps_out.tile([D, HW], F32, tag="out_ps")
        for cc in range(CC):
            nc.tensor.matmul(q_ps, _r(wq_sb[:, cc, :]), _r(xq_sb[:, cc, :]),
                             start=(cc == 0), stop=(cc == CC - 1))
        q_sb = consts.tile([D, HW], F32)
        nc.vector.tensor_copy(_r(q_sb), q_ps)

        qv_ps = ps_out.tile([D, HW], F32, tag="out_ps")
        for cc in range(CC):
            nc.tensor.matmul(qv_ps, _r(wv_sb[:, cc, :D]), _r(xq_sb[:, cc, :]),
                             start=(cc == 0), stop=(cc == CC - 1))
        qv_sb = consts.tile([D, HW], F32)
        nc.vector.tensor_copy(qv_sb, qv_ps)
        nc.sync.dma_start(out=qv_d, in_=qv_sb)

        # ---- per class-slot k ----
        for k in range(K):
            # supports slice: 4 x [128, 2000] (1 MB contiguous DMAs),
            # k=0 prefetched before the preamble
            if k in s_tiles:
                s_sb = s_tiles.pop(k)
            else:
                s_sb = [spool.tile([128, NIJ], F32, tag=f"s{cc}",
                                   name=f"s_sb{k}_{cc}") for cc in range(CC)]
                for cc in range(CC):
                    nc.sync.dma_start(out=_r(s_sb[cc]), in_=_r(s_d[k, cc]))

            # ks[d, nij] = Wqk @ s   (weight-stationary)
            ks_sb = kvpool.tile([128, NIJ], F32, tag="ks")
            for j in range(KSJ):
                ks_ps = ps_mm.tile([D, KSW], F32, tag="mm_ps")
                for cc in range(CC):
                    nc.tensor.matmul(ks_ps, _r(wq_sb[:, cc, :]),
                                     _r(s_sb[cc][:, j * KSW:(j + 1) * KSW]),
                                     start=(cc == 0), stop=(cc == CC - 1))
                nc.vector.tensor_copy(_r(ks_sb[:, j * KSW:(j + 1) * KSW]), ks_ps)

            # vT[nij, c] = (s^T) @ WvT  (s-chunks as weights -> transposed out)
            vt_sb = kvpool.tile([CH, NT, D], F32, tag="vt")
            for t in range(NT):
                vt_ps = ps_mm.tile([CH, DW], F32, tag="mm_ps")
                for cc in range(CC):
                    nc.tensor.matmul(vt_ps, _r(s_sb[cc][:, t * CH:(t + 1) * CH]),
                                     _r(wv_sb[:, cc, :]),
                                     start=(cc == 0), stop=(cc == CC - 1))
                nc.vector.tensor_copy(_r(vt_sb[:, t, :]), vt_ps[:, :D])

            # attention chunks: simT -> exp -> accumulate out and expsum
            o_ps = ps_out.tile([D, HW], F32, tag="out_ps")
            sum_ps = ps_sum.tile([1, HW], F32, tag="sum_ps")
            for t in range(NT):
                sim_ps = ps_sim.tile([CH, HW], F32, tag="sim_ps")
                nc.tensor.matmul(sim_ps, _r(ks_sb[:, t * CH:(t + 1) * CH]),
                                 _r(q_sb), start=True, stop=True)
                e_sb = epool.tile([CH, HW], F32, tag="e")
                nc.scalar.activation(_r(e_sb), sim_ps,
                                     mybir.ActivationFunctionType.Exp,
                                     scale=SCALE)
                nc.tensor.matmul(o_ps, _r(vt_sb[:, t, :]), _r(e_sb),
                                 start=(t == 0), stop=(t == NT - 1))
                # softmax denominator: ones-matmul partition reduction
                nc.tensor.matmul(sum_ps, _r(ones_col), _r(e_sb),
                                 start=(t == 0), stop=(t == NT - 1))

            recip = epool.tile([1, HW], F32, tag="recip")
            nc.vector.reciprocal(recip, sum_ps)
            # broadcast across partitions via outer product, then normalize
            bc_ps = ps_sim.tile([D, HW], F32, tag="sim_ps")
            nc.tensor.matmul(bc_ps, ones_row, recip, start=True, stop=True)
            bc_sb = epool.tile([D, HW], F32, tag="bc")
            nc.vector.tensor_copy(bc_sb, bc_ps)
            o_sb = opool.tile([D, HW], F32, tag="osb")
            nc.vector.tensor_mul(o_sb, o_ps, bc_sb)
            nc.sync.dma_start(out=out_d[k], in_=o_sb)

    nc.compile()
    return nc


def _get_nc():
    if "nc" not in _CACHE:
        _CACHE["nc"] = _build_program()
    return _CACHE["nc"]


def _prep_core_inputs(query_repr, supports_repr, wqkT, wvT, b):
    xq = np.ascontiguousarray(query_repr[b].reshape(CC, 128, HW))
    # supports [K,N,C,H,W] -> [K, CC, 128, N*HW] with C chunked on partitions
    s = supports_repr[b].reshape(K, N, CC, 128, HW)
    s = np.ascontiguousarray(s.transpose(0, 2, 3, 1, 4)).reshape(K, CC, 128, NIJ)
    return {"xq": xq, "s": s, "wqkT": wqkT, "wvT": wvT}


def kernel(query_repr, supports_repr, Wqk, Wv):
    query_repr = np.asarray(query_repr, dtype=np.float32)
    supports_repr = np.asarray(supports_repr, dtype=np.float32)
    wqkT = np.ascontiguousarray(np.asarray(Wqk, np.float32).T).reshape(CC, 128, D)
    wvT = np.ascontiguousarray(np.asarray(Wv, np.float32).T).reshape(CC, 128, D)

    nc = _get_nc()
    in_maps = [_prep_core_inputs(query_repr, supports_repr, wqkT, wvT, b)
               for b in range(B)]
    res = run_bass_kernel_spmd(nc, in_maps, list(range(B))).results

    query_v = np.stack([res[b]["qv"] for b in range(B)]).reshape(B, D, H, W)
    out = np.stack([res[b]["out"] for b in range(B)]).reshape(B, K, D, H, W)
    return query_v.astype(np.float32), out.astype(np.float32)


# revision 12
# speedup vs baseline: 2.0497x; 1.0853x over previous
"""Trainium2 Bass kernel for nn_CrossTransformer (cross-attention over support set).

Contract: kernel(**inputs) takes FULL inputs (query_repr [8,512,20,20],
supports_repr [8,5,5,512,20,20], Wqk [128,512], Wv [128,512]) and returns the
full outputs (query_v [8,128,20,20], out [8,5,128,20,20]) as float32, matching
reference():
    query_v = Wv @ query_repr                       (1x1 conv)
    sim     = (Wqk @ query)^T (Wqk @ supports) * 128^-0.5
    attn    = softmax over (n,i,j) of supports
    out     = attn @ (Wv @ supports)

Sharding: pure data-parallel over batch b -- each of the 8 NeuronCores handles
one batch element; no collectives.

Per-core strategy (everything stays transposed; no on-chip transposes needed):
  - matmuls run in float32r (single-pass fp32, 4x faster than fp32's LOW_HIGH
    two-pass mode; measured scale-rel err ~1.6e-4 vs 2.3e-3 for bf16). f32r
    needs moving-dim >= 256 for full rate, so the vT projection's rhs is
    padded from 128 to 256 columns (the extra output columns are never read).
  - sim^T [nij, hw] = ks^T q with lhsT = ks (natural), rhs = q (natural)
  - v^T [nij, c] produced directly by using s-chunks as the matmul weights
  - softmax over nij (the partition dim): exp on ScalarE (no max-subtraction
    needed: sim*scale ~ N(0,1), |max| ~ 7, far inside f32 exp range);
    denominator via GpSimd accumulation of exp chunks (keeps DVE free) + one
    ones-vector matmul for the partition reduction; normalization by an
    outer-product broadcast matmul + DVE multiply.
"""

import numpy as np
from contextlib import ExitStack

import concourse.bass as bass
import concourse.tile as tile
from concourse import bacc, masks, mybir
from concourse.bass_utils import run_bass_kernel_spmd

# Problem shape (hardcoded per harness contract)
B, K, N, C, D = 8, 5, 5, 512, 128
H = W = 20
HW = H * W                 # 400
NIJ = N * HW               # 2000
CC = C // 128              # 4 contraction chunks of 128
NT = 16                    # nij chunks
CH = NIJ // NT             # 125 rows per chunk
KSJ = 4                    # ks free-dim chunks
KSW = NIJ // KSJ           # 500 (<= 512 f32 moving-operand limit)
DW = 256                   # vT rhs padded width (f32r full rate needs >=256)
SCALE = float(D) ** -0.5
F32 = mybir.dt.float32
F32R = mybir.dt.float32r

_CACHE = {}


def _r(ap):
    """View an f32 AP as float32r for single-pass PE matmul."""
    return ap.bitcast(F32R)


def _build_program():
    nc = bacc.Bacc("TRN2", target_bir_lowering=False, debug=False)

    xq_d = nc.dram_tensor("xq", [CC, 128, HW], F32, kind="ExternalInput").ap()
    s_d = nc.dram_tensor("s", [K, CC, 128, NIJ], F32, kind="ExternalInput").ap()
    wq_d = nc.dram_tensor("wqkT", [CC, 128, D], F32, kind="ExternalInput").ap()
    wv_d = nc.dram_tensor("wvT", [CC, 128, D], F32, kind="ExternalInput").ap()
    qv_d = nc.dram_tensor("qv", [D, HW], F32, kind="ExternalOutput").ap()
    out_d = nc.dram_tensor("out", [K, D, HW], F32, kind="ExternalOutput").ap()

    with ExitStack() as ctx:
        tc = ctx.enter_context(tile.TileContext(nc))

        consts = ctx.enter_context(tc.tile_pool(name="consts", bufs=1))
        spool = ctx.enter_context(tc.tile_pool(name="spool", bufs=3))
        kvpool = ctx.enter_context(tc.tile_pool(name="kvpool", bufs=2))
        epool = ctx.enter_context(tc.tile_pool(name="epool", bufs=4))
        opool = ctx.enter_context(tc.tile_pool(name="opool", bufs=2))
        # PSUM budget (8 banks): mm_ps shared ks/vt rotation 3 + sim 3 + out 2
        ps_mm = ctx.enter_context(tc.tile_pool(name="ps_mm", bufs=3, space="PSUM"))
        ps_sim = ctx.enter_context(tc.tile_pool(name="ps_sim", bufs=2, space="PSUM"))
        ps_sum = ctx.enter_context(tc.tile_pool(name="ps_sum", bufs=1, space="PSUM"))
        ps_out = ctx.enter_context(tc.tile_pool(name="ps_out", bufs=2, space="PSUM"))

        def load_s(k):
            tiles = [spool.tile([128, NIJ], F32, tag=f"s{cc}",
                                name=f"s_sb{k}_{cc}") for cc in range(CC)]
            # j-granular 256 KB DMAs so the first ks group can start early
            for j in range(KSJ):
                for cc in range(CC):
                    nc.sync.dma_start(
                        out=_r(tiles[cc][:, j * KSW:(j + 1) * KSW]),
                        in_=_r(s_d[k, cc, :, j * KSW:(j + 1) * KSW]))
            return tiles

        # ---- prefetch first supports slice before anything else ----
        s_tiles = {0: load_s(0)}

        # ---- constants / per-batch tensors ----
        wq_sb = consts.tile([128, CC, D], F32)
        wv_sb = consts.tile([128, CC, DW], F32)   # [:, :, :D] real, rest pad
        xq_sb = consts.tile([128, CC, HW], F32)
        for cc in range(CC):
            nc.sync.dma_start(out=_r(wq_sb[:, cc, :]), in_=_r(wq_d[cc]))
            nc.sync.dma_start(out=_r(wv_sb[:, cc, :D]), in_=_r(wv_d[cc]))
            # pad cols [D:DW] with a second WvT copy: never read from the
            # vT psum, but f32r matmuls need all-f32r-typed producers
            nc.sync.dma_start(out=_r(wv_sb[:, cc, D:]), in_=_r(wv_d[cc]))
            nc.sync.dma_start(out=_r(xq_sb[:, cc, :]), in_=_r(xq_d[cc]))
        id_tmp = consts.tile([D, D], F32)
        masks.make_identity(nc, id_tmp)
        identity = consts.tile([D, D], F32)
        nc.vector.tensor_copy(_r(identity), id_tmp)

        ones_tmp = consts.tile([CH, 1], F32)
        nc.vector.memset(ones_tmp, 1.0)
        ones_col = consts.tile([CH, 1], F32)
        nc.vector.tensor_copy(_r(ones_col), ones_tmp)
        ones_row = consts.tile([1, D], F32)
        nc.vector.memset(ones_row, 1.0)

        # ---- q / qv projections (once per batch) ----
        q_ps = ps_out.tile([D, HW], F32, tag="out_ps")
        for cc in range(CC):
            nc.tensor.matmul(q_ps, _r(wq_sb[:, cc, :]), _r(xq_sb[:, cc, :]),
                             start=(cc == 0), stop=(cc == CC - 1))
        q_sb = consts.tile([D, HW], F32)
        nc.vector.tensor_copy(_r(q_sb), q_ps)

        qv_ps = ps_out.tile([D, HW], F32, tag="out_ps")
        for cc in range(CC):
            nc.tensor.matmul(qv_ps, _r(wv_sb[:, cc, :D]), _r(xq_sb[:, cc, :]),
                             start=(cc == 0), stop=(cc == CC - 1))
        qv_sb = consts.tile([D, HW], F32)
        nc.vector.tensor_copy(qv_sb, qv_ps)
        nc.sync.dma_start(out=qv_d, in_=qv_sb)

        # ---- per class-slot k ----
        for k in range(K):
            # supports slice: 4 x [128, 2000] (1 MB contiguous DMAs),
            # k=0 prefetched before the preamble
            s_sb = s_tiles.pop(k) if k in s_tiles else load_s(k)

            # ks[d, nij] = Wqk @ s   (weight-stationary)
            ks_sb = kvpool.tile([128, NIJ], F32, tag="ks")
            for j in range(KSJ):
                ks_ps = ps_mm.tile([D, KSW], F32, tag="mm_ps")
                for cc in range(CC):
                    nc.tensor.matmul(ks_ps, _r(wq_sb[:, cc, :]),
                                     _r(s_sb[cc][:, j * KSW:(j + 1) * KSW]),
                                     start=(cc == 0), stop=(cc == CC - 1))
                nc.vector.tensor_copy(_r(ks_sb[:, j * KSW:(j + 1) * KSW]), ks_ps)

            # v[c, nij] = Wv @ s (weight-stationary), then PE-transpose
            # 128x125 blocks to get vT[nij, c] for the out-matmul weights
            v_sb = kvpool.tile([D, NIJ], F32, tag="v")
            for j in range(KSJ):
                v_ps = ps_mm.tile([D, KSW], F32, tag="mm_ps")
                for cc in range(CC):
                    nc.tensor.matmul(v_ps, _r(wv_sb[:, cc, :D]),
                                     _r(s_sb[cc][:, j * KSW:(j + 1) * KSW]),
                                     start=(cc == 0), stop=(cc == CC - 1))
                nc.vector.tensor_copy(_r(v_sb[:, j * KSW:(j + 1) * KSW]), v_ps)
            vt_sb = kvpool.tile([CH, NT, D], F32, tag="vt")
            for t in range(NT):
                vt_ps = ps_mm.tile([CH, DW], F32, tag="mm_ps")
                nc.tensor.matmul(_r(vt_ps[:, :D]), _r(v_sb[:, t * CH:(t + 1) * CH]),
                                 _r(identity), is_transpose=True,
                                 start=True, stop=True)
                nc.vector.tensor_copy(_r(vt_sb[:, t, :]), vt_ps[:, :D])

            # attention chunks: simT -> exp -> accumulate out and expsum
            o_ps = ps_out.tile([D, HW], F32, tag="out_ps")
            sum_ps = ps_sum.tile([1, HW], F32, tag="sum_ps")
            LAG = 2   # run out/sum matmuls 2 chunks behind sim/exp so the
            e_tiles = {}  # PE queue never stalls on the exp of the same chunk
            for t in range(NT + LAG):
                if t < NT:
                    sim_ps = ps_sim.tile([CH, HW], F32, tag="sim_ps")
                    nc.tensor.matmul(sim_ps, _r(ks_sb[:, t * CH:(t + 1) * CH]),
                                     _r(q_sb), start=True, stop=True)
                    e_sb = epool.tile([CH, HW], F32, tag="e")
                    nc.scalar.activation(_r(e_sb), sim_ps,
                                         mybir.ActivationFunctionType.Exp,
                                         scale=SCALE)
                    e_tiles[t] = e_sb
                if t >= LAG:
                    td = t - LAG
                    e_sb = e_tiles.pop(td)
                    nc.tensor.matmul(o_ps, _r(vt_sb[:, td, :]), _r(e_sb),
                                     start=(td == 0), stop=(td == NT - 1))
                    # softmax denominator: ones-matmul partition reduction
                    nc.tensor.matmul(sum_ps, _r(ones_col), _r(e_sb),
                                     start=(td == 0), stop=(td == NT - 1))

            lns = epool.tile([1, HW], F32, tag="lns")
            nc.scalar.activation(lns, sum_ps, mybir.ActivationFunctionType.Ln)
            recip = epool.tile([1, HW], F32, tag="recip")
            nc.scalar.activation(recip, lns, mybir.ActivationFunctionType.Exp,
                                 scale=-1.0)
            # broadcast across partitions via outer product, then normalize
            bc_ps = ps_sim.tile([D, HW], F32, tag="sim_ps")
            nc.tensor.matmul(bc_ps, ones_row, recip, start=True, stop=True)
            bc_sb = epool.tile([D, HW], F32, tag="bc")
            nc.vector.tensor_copy(bc_sb, bc_ps)
            o_sb = opool.tile([D, HW], F32, tag="osb")
            nc.vector.tensor_mul(o_sb, o_ps, bc_sb)
            nc.sync.dma_start(out=out_d[k], in_=o_sb)

    nc.compile()
    return nc


def _get_nc():
    if "nc" not in _CACHE:
        _CACHE["nc"] = _build_program()
    return _CACHE["nc"]


def _prep_core_inputs(query_repr, supports_repr, wqkT, wvT, b):
    xq = np.ascontiguousarray(query_repr[b].reshape(CC, 128, HW))
    # supports [K,N,C,H,W] -> [K, CC, 128, N*HW] with C chunked on partitions
    s = supports_repr[b].reshape(K, N, CC, 128, HW)
    s = np.ascontiguousarray(s.transpose(0, 2, 3, 1, 4)).reshape(K, CC, 128, NIJ)
    return {"xq": xq, "s": s, "wqkT": wqkT, "wvT": wvT}


def kernel(query_repr, supports_repr, Wqk, Wv):
    query_repr = np.asarray(query_repr, dtype=np.float32)
    supports_repr = np.asarray(supports_repr, dtype=np.float32)
    wqkT = np.ascontiguousarray(np.asarray(Wqk, np.float32).T).reshape(CC, 128, D)
    wvT = np.ascontiguousarray(np.asarray(Wv, np.float32).T).reshape(CC, 128, D)

    nc = _get_nc()
    in_maps = [_prep_core_inputs(query_repr, supports_repr, wqkT, wvT, b)
               for b in range(B)]
    res = run_bass_kernel_spmd(nc, in_maps, list(range(B))).results

    query_v = np.stack([res[b]["qv"] for b in range(B)]).reshape(B, D, H, W)
    out = np.stack([res[b]["out"] for b in range(B)]).reshape(B, K, D, H, W)
    return query_v.astype(np.float32), out.astype(np.float32)


# revision 13
# speedup vs baseline: 2.0527x; 1.0015x over previous
"""Trainium2 Bass kernel for nn_CrossTransformer (cross-attention over support set).

Contract: kernel(**inputs) takes FULL inputs (query_repr [8,512,20,20],
supports_repr [8,5,5,512,20,20], Wqk [128,512], Wv [128,512]) and returns the
full outputs (query_v [8,128,20,20], out [8,5,128,20,20]) as float32, matching
reference():
    query_v = Wv @ query_repr                       (1x1 conv)
    sim     = (Wqk @ query)^T (Wqk @ supports) * 128^-0.5
    attn    = softmax over (n,i,j) of supports
    out     = attn @ (Wv @ supports)

Sharding: pure data-parallel over batch b -- each of the 8 NeuronCores handles
one batch element; no collectives.

Per-core strategy (everything stays transposed; no on-chip transposes needed):
  - matmuls run in float32r (single-pass fp32, 4x faster than fp32's LOW_HIGH
    two-pass mode; measured scale-rel err ~1.6e-4 vs 2.3e-3 for bf16). f32r
    needs moving-dim >= 256 for full rate, so the vT projection's rhs is
    padded from 128 to 256 columns (the extra output columns are never read).
  - sim^T [nij, hw] = ks^T q with lhsT = ks (natural), rhs = q (natural)
  - v^T [nij, c] produced directly by using s-chunks as the matmul weights
  - softmax over nij (the partition dim): exp on ScalarE (no max-subtraction
    needed: sim*scale ~ N(0,1), |max| ~ 7, far inside f32 exp range);
    denominator via GpSimd accumulation of exp chunks (keeps DVE free) + one
    ones-vector matmul for the partition reduction; normalization by an
    outer-product broadcast matmul + DVE multiply.
"""

import numpy as np
from contextlib import ExitStack

import concourse.bass as bass
import concourse.tile as tile
from concourse import bacc, masks, mybir
from concourse.bass_utils import run_bass_kernel_spmd

# Problem shape (hardcoded per harness contract)
B, K, N, C, D = 8, 5, 5, 512, 128
H = W = 20
HW = H * W                 # 400
NIJ = N * HW               # 2000
CC = C // 128              # 4 contraction chunks of 128
NT = 16                    # nij chunks
CH = NIJ // NT             # 125 rows per chunk
KSJ = 4                    # ks free-dim chunks
KSW = NIJ // KSJ           # 500 (<= 512 f32 moving-operand limit)
DW = 256                   # vT rhs padded width (f32r full rate needs >=256)
SCALE = float(D) ** -0.5
F32 = mybir.dt.float32
F32R = mybir.dt.float32r

_CACHE = {}


def _r(ap):
    """View an f32 AP as float32r for single-pass PE matmul."""
    return ap.bitcast(F32R)


def _build_program():
    nc = bacc.Bacc("TRN2", target_bir_lowering=False, debug=False)

    xq_d = nc.dram_tensor("xq", [CC, 128, HW], F32, kind="ExternalInput").ap()
    s_d = nc.dram_tensor("s", [K, CC, 128, NIJ], F32, kind="ExternalInput").ap()
    wq_d = nc.dram_tensor("wqkT", [CC, 128, D], F32, kind="ExternalInput").ap()
    wv_d = nc.dram_tensor("wvT", [CC, 128, D], F32, kind="ExternalInput").ap()
    qv_d = nc.dram_tensor("qv", [D, HW], F32, kind="ExternalOutput").ap()
    out_d = nc.dram_tensor("out", [K, D, HW], F32, kind="ExternalOutput").ap()

    with ExitStack() as ctx:
        tc = ctx.enter_context(tile.TileContext(nc))

        consts = ctx.enter_context(tc.tile_pool(name="consts", bufs=1))
        spool = ctx.enter_context(tc.tile_pool(name="spool", bufs=3))
        kvpool = ctx.enter_context(tc.tile_pool(name="kvpool", bufs=2))
        epool = ctx.enter_context(tc.tile_pool(name="epool", bufs=4))
        opool = ctx.enter_context(tc.tile_pool(name="opool", bufs=2))
        # PSUM budget (8 banks): mm_ps shared ks/vt rotation 3 + sim 3 + out 2
        ps_mm = ctx.enter_context(tc.tile_pool(name="ps_mm", bufs=3, space="PSUM"))
        ps_sim = ctx.enter_context(tc.tile_pool(name="ps_sim", bufs=2, space="PSUM"))
        ps_sum = ctx.enter_context(tc.tile_pool(name="ps_sum", bufs=1, space="PSUM"))
        ps_out = ctx.enter_context(tc.tile_pool(name="ps_out", bufs=2, space="PSUM"))

        def load_s(k, granular=False):
            tiles = [spool.tile([128, NIJ], F32, tag=f"s{cc}",
                                name=f"s_sb{k}_{cc}") for cc in range(CC)]
            if granular:
                # j-granular 256 KB DMAs so the first ks group starts early
                for j in range(KSJ):
                    for cc in range(CC):
                        nc.sync.dma_start(
                            out=_r(tiles[cc][:, j * KSW:(j + 1) * KSW]),
                            in_=_r(s_d[k, cc, :, j * KSW:(j + 1) * KSW]))
            else:
                for cc in range(CC):
                    nc.sync.dma_start(out=_r(tiles[cc]), in_=_r(s_d[k, cc]))
            return tiles

        # ---- small inputs first (unblock q/qv), then first supports slice
        wq_sb = consts.tile([128, CC, D], F32)
        wv_sb = consts.tile([128, CC, DW], F32)   # [:, :, :D] real, rest pad
        xq_sb = consts.tile([128, CC, HW], F32)
        for cc in range(CC):
            nc.sync.dma_start(out=_r(xq_sb[:, cc, :]), in_=_r(xq_d[cc]))
            nc.sync.dma_start(out=_r(wq_sb[:, cc, :]), in_=_r(wq_d[cc]))
            nc.sync.dma_start(out=_r(wv_sb[:, cc, :D]), in_=_r(wv_d[cc]))
            # pad cols [D:DW] with a second WvT copy: never read from the
            # vT psum, but f32r matmuls need all-f32r-typed producers
            nc.sync.dma_start(out=_r(wv_sb[:, cc, D:]), in_=_r(wv_d[cc]))
        s_tiles = {0: load_s(0, granular=True)}
        id_tmp = consts.tile([D, D], F32)
        masks.make_identity(nc, id_tmp)
        identity = consts.tile([D, D], F32)
        nc.vector.tensor_copy(_r(identity), id_tmp)

        ones_tmp = consts.tile([CH, 1], F32)
        nc.vector.memset(ones_tmp, 1.0)
        ones_col = consts.tile([CH, 1], F32)
        nc.vector.tensor_copy(_r(ones_col), ones_tmp)
        ones_row = consts.tile([1, D], F32)
        nc.vector.memset(ones_row, 1.0)

        # ---- q / qv projections (once per batch) ----
        q_ps = ps_out.tile([D, HW], F32, tag="out_ps")
        for cc in range(CC):
            nc.tensor.matmul(q_ps, _r(wq_sb[:, cc, :]), _r(xq_sb[:, cc, :]),
                             start=(cc == 0), stop=(cc == CC - 1))
        q_sb = consts.tile([D, HW], F32)
        nc.vector.tensor_copy(_r(q_sb), q_ps)

        qv_ps = ps_out.tile([D, HW], F32, tag="out_ps")
        for cc in range(CC):
            nc.tensor.matmul(qv_ps, _r(wv_sb[:, cc, :D]), _r(xq_sb[:, cc, :]),
                             start=(cc == 0), stop=(cc == CC - 1))
        qv_sb = consts.tile([D, HW], F32)
        nc.vector.tensor_copy(qv_sb, qv_ps)
        nc.sync.dma_start(out=qv_d, in_=qv_sb)

        # ---- per class-slot k ----
        pending_tail = None
        for k in range(K):
            # supports slice: 4 x [128, 2000] (1 MB contiguous DMAs),
            # k=0 prefetched before the preamble
            s_sb = s_tiles.pop(k) if k in s_tiles else load_s(k)

            # ks[d, nij] = Wqk @ s   (weight-stationary)
            ks_sb = kvpool.tile([128, NIJ], F32, tag="ks")
            for j in range(KSJ):
                ks_ps = ps_mm.tile([D, KSW], F32, tag="mm_ps")
                for cc in range(CC):
                    nc.tensor.matmul(ks_ps, _r(wq_sb[:, cc, :]),
                                     _r(s_sb[cc][:, j * KSW:(j + 1) * KSW]),
                                     start=(cc == 0), stop=(cc == CC - 1))
                nc.vector.tensor_copy(_r(ks_sb[:, j * KSW:(j + 1) * KSW]), ks_ps)

            # v[c, nij] = Wv @ s (weight-stationary), then PE-transpose
            # 128x125 blocks to get vT[nij, c] for the out-matmul weights
            v_sb = kvpool.tile([D, NIJ], F32, tag="v")
            for j in range(KSJ):
                v_ps = ps_mm.tile([D, KSW], F32, tag="mm_ps")
                for cc in range(CC):
                    nc.tensor.matmul(v_ps, _r(wv_sb[:, cc, :D]),
                                     _r(s_sb[cc][:, j * KSW:(j + 1) * KSW]),
                                     start=(cc == 0), stop=(cc == CC - 1))
                nc.vector.tensor_copy(_r(v_sb[:, j * KSW:(j + 1) * KSW]), v_ps)
            vt_sb = kvpool.tile([CH, NT, D], F32, tag="vt")
            for t in range(NT):
                vt_ps = ps_mm.tile([CH, DW], F32, tag="mm_ps")
                nc.tensor.matmul(_r(vt_ps[:, :D]), _r(v_sb[:, t * CH:(t + 1) * CH]),
                                 _r(identity), is_transpose=True,
                                 start=True, stop=True)
                nc.vector.tensor_copy(_r(vt_sb[:, t, :]), vt_ps[:, :D])

            # emit previous k's normalization tail now: its bc-matmul lands
            # behind this k's projection matmuls in the in-order PE queue, so
            # the reciprocal chain resolves without stalling the PE
            if pending_tail is not None:
                pending_tail()
                pending_tail = None

            # attention chunks: simT -> exp -> accumulate out and expsum
            o_ps = ps_out.tile([D, HW], F32, tag="out_ps")
            sum_ps = ps_sum.tile([1, HW], F32, tag="sum_ps")
            LAG = 2   # run out/sum matmuls 2 chunks behind sim/exp so the
            e_tiles = {}  # PE queue never stalls on the exp of the same chunk
            for t in range(NT + LAG):
                if t < NT:
                    sim_ps = ps_sim.tile([CH, HW], F32, tag="sim_ps")
                    nc.tensor.matmul(sim_ps, _r(ks_sb[:, t * CH:(t + 1) * CH]),
                                     _r(q_sb), start=True, stop=True)
                    e_sb = epool.tile([CH, HW], F32, tag="e")
                    nc.scalar.activation(_r(e_sb), sim_ps,
                                         mybir.ActivationFunctionType.Exp,
                                         scale=SCALE)
                    e_tiles[t] = e_sb
                if t >= LAG:
                    td = t - LAG
                    e_sb = e_tiles.pop(td)
                    nc.tensor.matmul(o_ps, _r(vt_sb[:, td, :]), _r(e_sb),
                                     start=(td == 0), stop=(td == NT - 1))
                    # softmax denominator: ones-matmul partition reduction
                    nc.tensor.matmul(sum_ps, _r(ones_col), _r(e_sb),
                                     start=(td == 0), stop=(td == NT - 1))

            def make_tail(k=k, o_ps=o_ps, sum_ps=sum_ps):
                def tail():
                    recip = epool.tile([1, HW], F32, tag="recip")
                    nc.vector.reciprocal(recip, sum_ps)
                    # partition-broadcast via outer product, then normalize
                    bc_ps = ps_sim.tile([D, HW], F32, tag="sim_ps")
                    nc.tensor.matmul(bc_ps, ones_row, recip,
                                     start=True, stop=True)
                    bc_sb = epool.tile([D, HW], F32, tag="bc")
                    nc.vector.tensor_copy(bc_sb, bc_ps)
                    o_sb = opool.tile([D, HW], F32, tag="osb")
                    nc.vector.tensor_mul(o_sb, o_ps, bc_sb)
                    nc.sync.dma_start(out=out_d[k], in_=o_sb)
                return tail
            pending_tail = make_tail()

        pending_tail()

    nc.compile()
    return nc


def _get_nc():
    if "nc" not in _CACHE:
        _CACHE["nc"] = _build_program()
    return _CACHE["nc"]


def _prep_core_inputs(query_repr, supports_repr, wqkT, wvT, b):
    xq = np.ascontiguousarray(query_repr[b].reshape(CC, 128, HW))
    # supports [K,N,C,H,W] -> [K, CC, 128, N*HW] with C chunked on partitions
    s = supports_repr[b].reshape(K, N, CC, 128, HW)
    s = np.ascontiguousarray(s.transpose(0, 2, 3, 1, 4)).reshape(K, CC, 128, NIJ)
    return {"xq": xq, "s": s, "wqkT": wqkT, "wvT": wvT}


def kernel(query_repr, supports_repr, Wqk, Wv):
    query_repr = np.asarray(query_repr, dtype=np.float32)
    supports_repr = np.asarray(supports_repr, dtype=np.float32)
    wqkT = np.ascontiguousarray(np.asarray(Wqk, np.float32).T).reshape(CC, 128, D)
    wvT = np.ascontiguousarray(np.asarray(Wv, np.float32).T).reshape(CC, 128, D)

    nc = _get_nc()
    in_maps = [_prep_core_inputs(query_repr, supports_repr, wqkT, wvT, b)
               for b in range(B)]
    res = run_bass_kernel_spmd(nc, in_maps, list(range(B))).results

    query_v = np.stack([res[b]["qv"] for b in range(B)]).reshape(B, D, H, W)
    out = np.stack([res[b]["out"] for b in range(B)]).reshape(B, K, D, H, W)
    return query_v.astype(np.float32), out.astype(np.float32)


# revision 15
# speedup vs baseline: 2.1636x; 1.0540x over previous
"""Trainium2 Bass kernel for nn_CrossTransformer (cross-attention over support set).

Contract: kernel(**inputs) takes FULL inputs (query_repr [8,512,20,20],
supports_repr [8,5,5,512,20,20], Wqk [128,512], Wv [128,512]) and returns the
full outputs (query_v [8,128,20,20], out [8,5,128,20,20]) as float32, matching
reference():
    query_v = Wv @ query_repr                       (1x1 conv)
    sim     = (Wqk @ query)^T (Wqk @ supports) * 128^-0.5
    attn    = softmax over (n,i,j) of supports
    out     = attn @ (Wv @ supports)

Sharding: pure data-parallel over batch b -- each of the 8 NeuronCores handles
one batch element; no collectives.

Per-core strategy (everything stays transposed; no on-chip transposes needed):
  - matmuls run in float32r (single-pass fp32, 4x faster than fp32's LOW_HIGH
    two-pass mode; measured scale-rel err ~1.6e-4 vs 2.3e-3 for bf16). f32r
    needs moving-dim >= 256 for full rate, so the vT projection's rhs is
    padded from 128 to 256 columns (the extra output columns are never read).
  - sim^T [nij, hw] = ks^T q with lhsT = ks (natural), rhs = q (natural)
  - v^T [nij, c] produced directly by using s-chunks as the matmul weights
  - softmax over nij (the partition dim): exp on ScalarE (no max-subtraction
    needed: sim*scale ~ N(0,1), |max| ~ 7, far inside f32 exp range);
    denominator via GpSimd accumulation of exp chunks (keeps DVE free) + one
    ones-vector matmul for the partition reduction; normalization by an
    outer-product broadcast matmul + DVE multiply.
"""

import numpy as np
from contextlib import ExitStack

import concourse.bass as bass
import concourse.tile as tile
from concourse import bacc, masks, mybir
from concourse.bass_utils import run_bass_kernel_spmd

# Problem shape (hardcoded per harness contract)
B, K, N, C, D = 8, 5, 5, 512, 128
H = W = 20
HW = H * W                 # 400
NIJ = N * HW               # 2000
CC = C // 128              # 4 contraction chunks of 128
NT = 16                    # nij chunks
CH = NIJ // NT             # 125 rows per chunk
KSJ = 4                    # ks free-dim chunks
KSW = NIJ // KSJ           # 500 (<= 512 f32 moving-operand limit)
DW = 256                   # vT rhs padded width (f32r full rate needs >=256)
SCALE = float(D) ** -0.5
F32 = mybir.dt.float32
F32R = mybir.dt.float32r

_CACHE = {}


def _r(ap):
    """View an f32 AP as float32r for single-pass PE matmul."""
    return ap.bitcast(F32R)


def _build_program():
    nc = bacc.Bacc("TRN2", target_bir_lowering=False, debug=False)

    xq_d = nc.dram_tensor("xq", [CC, 128, HW], F32, kind="ExternalInput").ap()
    s_d = nc.dram_tensor("s", [K, CC, 128, NIJ], F32, kind="ExternalInput").ap()
    wq_d = nc.dram_tensor("wqkT", [CC, 128, D], F32, kind="ExternalInput").ap()
    wv_d = nc.dram_tensor("wvT", [CC, 128, DW], F32, kind="ExternalInput").ap()
    qv_d = nc.dram_tensor("qv", [D, HW], F32, kind="ExternalOutput").ap()
    out_d = nc.dram_tensor("out", [K, D, HW], F32, kind="ExternalOutput").ap()

    with ExitStack() as ctx:
        tc = ctx.enter_context(tile.TileContext(nc))

        consts = ctx.enter_context(tc.tile_pool(name="consts", bufs=1))
        spool = ctx.enter_context(tc.tile_pool(name="spool", bufs=3))
        kvpool = ctx.enter_context(tc.tile_pool(name="kvpool", bufs=2))
        epool = ctx.enter_context(tc.tile_pool(name="epool", bufs=6))
        opool = ctx.enter_context(tc.tile_pool(name="opool", bufs=2))
        # PSUM budget (8 banks): mm_ps shared ks/vt rotation 3 + sim 3 + out 2
        ps_mm = ctx.enter_context(tc.tile_pool(name="ps_mm", bufs=3, space="PSUM"))
        ps_sim = ctx.enter_context(tc.tile_pool(name="ps_sim", bufs=2, space="PSUM"))
        ps_sum = ctx.enter_context(tc.tile_pool(name="ps_sum", bufs=1, space="PSUM"))
        ps_out = ctx.enter_context(tc.tile_pool(name="ps_out", bufs=2, space="PSUM"))

        def load_s(k, granular=False):
            tiles = [spool.tile([128, NIJ], F32, tag=f"s{cc}",
                                name=f"s_sb{k}_{cc}") for cc in range(CC)]
            if granular:
                # j-granular 256 KB DMAs so the first ks group starts early
                for j in range(KSJ):
                    for cc in range(CC):
                        nc.sync.dma_start(
                            out=_r(tiles[cc][:, j * KSW:(j + 1) * KSW]),
                            in_=_r(s_d[k, cc, :, j * KSW:(j + 1) * KSW]))
            else:
                for cc in range(CC):
                    nc.sync.dma_start(out=_r(tiles[cc]), in_=_r(s_d[k, cc]))
            return tiles

        # ---- small inputs first (unblock q/qv), then first supports slice
        wq_sb = consts.tile([128, CC, D], F32)
        wv_sb = consts.tile([128, CC, DW], F32)   # [:, :, :D] real, rest pad
        xq_sb = consts.tile([128, CC, HW], F32)
        # issue small inputs on the scalar HWDGE ring so they don't queue
        # behind the 1 MB supports loads on the sync ring
        for cc in range(CC):
            nc.scalar.dma_start(out=_r(xq_sb[:, cc, :]), in_=_r(xq_d[cc]))
            nc.scalar.dma_start(out=_r(wq_sb[:, cc, :]), in_=_r(wq_d[cc]))
            # wvT arrives host-padded to DW cols (f32r needs >=256 moving dim)
            nc.scalar.dma_start(out=_r(wv_sb[:, cc, :]), in_=_r(wv_d[cc]))
        s_tiles = {0: load_s(0, granular=True)}
        id_tmp = consts.tile([D, D], F32)
        masks.make_identity(nc, id_tmp)
        identity = consts.tile([D, D], F32)
        nc.vector.tensor_copy(_r(identity), id_tmp)

        ones_tmp = consts.tile([CH, 1], F32)
        nc.vector.memset(ones_tmp, 1.0)
        ones_col = consts.tile([CH, 1], F32)
        nc.vector.tensor_copy(_r(ones_col), ones_tmp)
        ones_row = consts.tile([1, D], F32)
        nc.vector.memset(ones_row, 1.0)

        # ---- q / qv projections (once per batch) ----
        q_ps = ps_out.tile([D, HW], F32, tag="out_ps")
        for cc in range(CC):
            nc.tensor.matmul(q_ps, _r(wq_sb[:, cc, :]), _r(xq_sb[:, cc, :]),
                             start=(cc == 0), stop=(cc == CC - 1))
        q_sb = consts.tile([D, HW], F32)
        nc.vector.tensor_copy(_r(q_sb), q_ps)

        qv_ps = ps_out.tile([D, HW], F32, tag="out_ps")
        for cc in range(CC):
            nc.tensor.matmul(qv_ps, _r(wv_sb[:, cc, :D]), _r(xq_sb[:, cc, :]),
                             start=(cc == 0), stop=(cc == CC - 1))
        qv_sb = consts.tile([D, HW], F32)
        nc.vector.tensor_copy(qv_sb, qv_ps)
        nc.scalar.dma_start(out=qv_d, in_=qv_sb)

        # ---- per class-slot k ----
        pending_tail = None
        for k in range(K):
            # supports slice: 4 x [128, 2000] (1 MB contiguous DMAs),
            # k=0 prefetched before the preamble
            s_sb = s_tiles.pop(k) if k in s_tiles else load_s(k)

            # ks[d, nij] = Wqk @ s   (weight-stationary)
            ks_sb = kvpool.tile([128, NIJ], F32, tag="ks")
            for j in range(KSJ):
                ks_ps = ps_mm.tile([D, KSW], F32, tag="mm_ps")
                for cc in range(CC):
                    nc.tensor.matmul(ks_ps, _r(wq_sb[:, cc, :]),
                                     _r(s_sb[cc][:, j * KSW:(j + 1) * KSW]),
                                     start=(cc == 0), stop=(cc == CC - 1))
                nc.vector.tensor_copy(_r(ks_sb[:, j * KSW:(j + 1) * KSW]), ks_ps)

            # v[c, nij] = Wv @ s (weight-stationary), then PE-transpose
            # 128x125 blocks to get vT[nij, c] for the out-matmul weights
            v_sb = kvpool.tile([D, NIJ], F32, tag="v")
            for j in range(KSJ):
                v_ps = ps_mm.tile([D, KSW], F32, tag="mm_ps")
                for cc in range(CC):
                    nc.tensor.matmul(v_ps, _r(wv_sb[:, cc, :D]),
                                     _r(s_sb[cc][:, j * KSW:(j + 1) * KSW]),
                                     start=(cc == 0), stop=(cc == CC - 1))
                nc.vector.tensor_copy(_r(v_sb[:, j * KSW:(j + 1) * KSW]), v_ps)
            vt_sb = kvpool.tile([CH, NT, D], F32, tag="vt")
            for t in range(NT):
                vt_ps = ps_mm.tile([CH, DW], F32, tag="mm_ps")
                nc.tensor.matmul(_r(vt_ps[:, :D]), _r(v_sb[:, t * CH:(t + 1) * CH]),
                                 _r(identity), is_transpose=True,
                                 start=True, stop=True)
                nc.vector.tensor_copy(_r(vt_sb[:, t, :]), vt_ps[:, :D])

            # emit previous k's normalization tail now: its bc-matmul lands
            # behind this k's projection matmuls in the in-order PE queue, so
            # the reciprocal chain resolves without stalling the PE
            if pending_tail is not None:
                pending_tail()
                pending_tail = None

            # attention chunks: simT -> exp -> accumulate out and expsum
            o_ps = ps_out.tile([D, HW], F32, tag="out_ps")
            sum_ps = ps_sum.tile([1, HW], F32, tag="sum_ps")
            LAG = 3   # run out/sum matmuls 3 chunks behind sim/exp so the
            e_tiles = {}  # PE queue never stalls on the exp of the same chunk
            for t in range(NT + LAG):
                if t < NT:
                    sim_ps = ps_sim.tile([CH, HW], F32, tag="sim_ps")
                    nc.tensor.matmul(sim_ps, _r(ks_sb[:, t * CH:(t + 1) * CH]),
                                     _r(q_sb), start=True, stop=True)
                    e_sb = epool.tile([CH, HW], F32, tag="e")
                    nc.scalar.activation(_r(e_sb), sim_ps,
                                         mybir.ActivationFunctionType.Exp,
                                         scale=SCALE)
                    e_tiles[t] = e_sb
                if t >= LAG:
                    td = t - LAG
                    e_sb = e_tiles.pop(td)
                    nc.tensor.matmul(o_ps, _r(vt_sb[:, td, :]), _r(e_sb),
                                     start=(td == 0), stop=(td == NT - 1))
                    # softmax denominator: ones-matmul partition reduction
                    nc.tensor.matmul(sum_ps, _r(ones_col), _r(e_sb),
                                     start=(td == 0), stop=(td == NT - 1))

            recip = epool.tile([1, HW], F32, tag="recip")
            nc.vector.reciprocal(recip, sum_ps)

            def make_tail(k=k, o_ps=o_ps, recip=recip):
                def tail():
                    # partition-broadcast via outer product, then normalize
                    bc_ps = ps_sim.tile([D, HW], F32, tag="sim_ps")
                    nc.tensor.matmul(bc_ps, ones_row, recip,
                                     start=True, stop=True)
                    bc_sb = epool.tile([D, HW], F32, tag="bc")
                    nc.vector.tensor_copy(bc_sb, bc_ps)
                    o_sb = opool.tile([D, HW], F32, tag="osb")
                    nc.vector.tensor_mul(o_sb, o_ps, bc_sb)
                    nc.scalar.dma_start(out=out_d[k], in_=o_sb)
                return tail
            pending_tail = make_tail()

        pending_tail()

    nc.compile()
    return nc


def _get_nc():
    if "nc" not in _CACHE:
        _CACHE["nc"] = _build_program()
    return _CACHE["nc"]


def _prep_weights(Wqk, Wv):
    wqkT = np.ascontiguousarray(np.asarray(Wqk, np.float32).T).reshape(CC, 128, D)
    wvT = np.ascontiguousarray(np.asarray(Wv, np.float32).T).reshape(CC, 128, D)
    # pad to DW cols (f32r full-rate needs moving dim >= 256; pad never read)
    wvT = np.ascontiguousarray(np.concatenate([wvT, wvT], axis=2))
    return wqkT, wvT


def _prep_core_inputs(query_repr, supports_repr, wqkT, wvT, b):
    xq = np.ascontiguousarray(query_repr[b].reshape(CC, 128, HW))
    # supports [K,N,C,H,W] -> [K, CC, 128, N*HW] with C chunked on partitions
    s = supports_repr[b].reshape(K, N, CC, 128, HW)
    s = np.ascontiguousarray(s.transpose(0, 2, 3, 1, 4)).reshape(K, CC, 128, NIJ)
    return {"xq": xq, "s": s, "wqkT": wqkT, "wvT": wvT}


def kernel(query_repr, supports_repr, Wqk, Wv):
    query_repr = np.asarray(query_repr, dtype=np.float32)
    supports_repr = np.asarray(supports_repr, dtype=np.float32)
    wqkT, wvT = _prep_weights(Wqk, Wv)

    nc = _get_nc()
    in_maps = [_prep_core_inputs(query_repr, supports_repr, wqkT, wvT, b)
               for b in range(B)]
    res = run_bass_kernel_spmd(nc, in_maps, list(range(B))).results

    query_v = np.stack([res[b]["qv"] for b in range(B)]).reshape(B, D, H, W)
    out = np.stack([res[b]["out"] for b in range(B)]).reshape(B, K, D, H, W)
    return query_v.astype(np.float32), out.astype(np.float32)


# revision 16
# speedup vs baseline: 2.2282x; 1.0299x over previous
"""Trainium2 Bass kernel for nn_CrossTransformer (cross-attention over support set).

Contract: kernel(**inputs) takes FULL inputs (query_repr [8,512,20,20],
supports_repr [8,5,5,512,20,20], Wqk [128,512], Wv [128,512]) and returns the
full outputs (query_v [8,128,20,20], out [8,5,128,20,20]) as float32, matching
reference():
    query_v = Wv @ query_repr                       (1x1 conv)
    sim     = (Wqk @ query)^T (Wqk @ supports) * 128^-0.5
    attn    = softmax over (n,i,j) of supports
    out     = attn @ (Wv @ supports)

Sharding: pure data-parallel over batch b -- each of the 8 NeuronCores handles
one batch element; no collectives.

Per-core strategy (everything stays transposed; no on-chip transposes needed):
  - matmuls run in float32r (single-pass fp32, 4x faster than fp32's LOW_HIGH
    two-pass mode; measured scale-rel err ~1.6e-4 vs 2.3e-3 for bf16). f32r
    needs moving-dim >= 256 for full rate, so the vT projection's rhs is
    padded from 128 to 256 columns (the extra output columns are never read).
  - sim^T [nij, hw] = ks^T q with lhsT = ks (natural), rhs = q (natural)
  - v^T [nij, c] produced directly by using s-chunks as the matmul weights
  - softmax over nij (the partition dim): exp on ScalarE (no max-subtraction
    needed: sim*scale ~ N(0,1), |max| ~ 7, far inside f32 exp range);
    denominator via GpSimd accumulation of exp chunks (keeps DVE free) + one
    ones-vector matmul for the partition reduction; normalization by an
    outer-product broadcast matmul + DVE multiply.
"""

import numpy as np
from contextlib import ExitStack

import concourse.bass as bass
import concourse.tile as tile
from concourse import bacc, masks, mybir
from concourse.bass_utils import run_bass_kernel_spmd

# Problem shape (hardcoded per harness contract)
B, K, N, C, D = 8, 5, 5, 512, 128
H = W = 20
HW = H * W                 # 400
NIJ = N * HW               # 2000
CC = C // 128              # 4 contraction chunks of 128
NT = 16                    # nij chunks
CH = NIJ // NT             # 125 rows per chunk
KSJ = 4                    # ks free-dim chunks
KSW = NIJ // KSJ           # 500 (<= 512 f32 moving-operand limit)
DW = 256                   # vT rhs padded width (f32r full rate needs >=256)
SCALE = float(D) ** -0.5
F32 = mybir.dt.float32
F32R = mybir.dt.float32r

_CACHE = {}


def _r(ap):
    """View an f32 AP as float32r for single-pass PE matmul."""
    return ap.bitcast(F32R)


def _build_program():
    nc = bacc.Bacc("TRN2", target_bir_lowering=False, debug=False)

    xq_d = nc.dram_tensor("xq", [CC, 128, HW], F32, kind="ExternalInput").ap()
    s_d = nc.dram_tensor("s", [K, CC, 128, NIJ], F32, kind="ExternalInput").ap()
    wq_d = nc.dram_tensor("wqkT", [CC, 128, D], F32, kind="ExternalInput").ap()
    wv_d = nc.dram_tensor("wvT", [CC, 128, DW], F32, kind="ExternalInput").ap()
    qv_d = nc.dram_tensor("qv", [D, HW], F32, kind="ExternalOutput").ap()
    out_d = nc.dram_tensor("out", [K, D, HW], F32, kind="ExternalOutput").ap()

    with ExitStack() as ctx:
        tc = ctx.enter_context(tile.TileContext(nc))

        consts = ctx.enter_context(tc.tile_pool(name="consts", bufs=1))
        spool = ctx.enter_context(tc.tile_pool(name="spool", bufs=3))
        kvpool = ctx.enter_context(tc.tile_pool(name="kvpool", bufs=2))
        epool = ctx.enter_context(tc.tile_pool(name="epool", bufs=6))
        opool = ctx.enter_context(tc.tile_pool(name="opool", bufs=2))
        # PSUM budget (8 banks): mm_ps shared ks/vt rotation 3 + sim 3 + out 2
        ps_mm = ctx.enter_context(tc.tile_pool(name="ps_mm", bufs=3, space="PSUM"))
        ps_sim = ctx.enter_context(tc.tile_pool(name="ps_sim", bufs=2, space="PSUM"))
        ps_sum = ctx.enter_context(tc.tile_pool(name="ps_sum", bufs=1, space="PSUM"))
        ps_out = ctx.enter_context(tc.tile_pool(name="ps_out", bufs=2, space="PSUM"))

        def load_s(k, granular=False):
            tiles = [spool.tile([128, NIJ], F32, tag=f"s{cc}",
                                name=f"s_sb{k}_{cc}") for cc in range(CC)]
            if granular:
                # j-granular 256 KB DMAs so the first ks group starts early
                for j in range(KSJ):
                    for cc in range(CC):
                        nc.sync.dma_start(
                            out=_r(tiles[cc][:, j * KSW:(j + 1) * KSW]),
                            in_=_r(s_d[k, cc, :, j * KSW:(j + 1) * KSW]))
            else:
                for cc in range(CC):
                    nc.sync.dma_start(out=_r(tiles[cc]), in_=_r(s_d[k, cc]))
            return tiles

        # ---- small inputs first (unblock q/qv), then first supports slice
        wq_sb = consts.tile([128, CC, D], F32)
        wv_sb = consts.tile([128, CC, DW], F32)   # [:, :, :D] real, rest pad
        xq_sb = consts.tile([128, CC, HW], F32)
        # issue small inputs on the scalar HWDGE ring so they don't queue
        # behind the 1 MB supports loads on the sync ring
        for cc in range(CC):
            nc.scalar.dma_start(out=_r(xq_sb[:, cc, :]), in_=_r(xq_d[cc]))
            nc.scalar.dma_start(out=_r(wq_sb[:, cc, :]), in_=_r(wq_d[cc]))
            # wvT arrives host-padded to DW cols (f32r needs >=256 moving dim)
            nc.scalar.dma_start(out=_r(wv_sb[:, cc, :]), in_=_r(wv_d[cc]))
        s_tiles = {0: load_s(0, granular=True)}
        id_tmp = consts.tile([D, D], F32)
        masks.make_identity(nc, id_tmp)
        identity = consts.tile([D, D], F32)
        nc.vector.tensor_copy(_r(identity), id_tmp)

        ones_tmp = consts.tile([CH, 1], F32)
        nc.vector.memset(ones_tmp, 1.0)
        ones_col = consts.tile([CH, 1], F32)
        nc.vector.tensor_copy(_r(ones_col), ones_tmp)
        ones_row = consts.tile([1, D], F32)
        nc.vector.memset(ones_row, 1.0)

        # ---- q / qv projections (once per batch) ----
        q_ps = ps_out.tile([D, HW], F32, tag="out_ps")
        for cc in range(CC):
            nc.tensor.matmul(q_ps, _r(wq_sb[:, cc, :]), _r(xq_sb[:, cc, :]),
                             start=(cc == 0), stop=(cc == CC - 1))
        q_sb = consts.tile([D, HW], F32)
        nc.vector.tensor_copy(_r(q_sb), q_ps)

        qv_ps = ps_out.tile([D, HW], F32, tag="out_ps")
        for cc in range(CC):
            nc.tensor.matmul(qv_ps, _r(wv_sb[:, cc, :D]), _r(xq_sb[:, cc, :]),
                             start=(cc == 0), stop=(cc == CC - 1))
        qv_sb = consts.tile([D, HW], F32)
        nc.vector.tensor_copy(qv_sb, qv_ps)
        nc.scalar.dma_start(out=qv_d, in_=qv_sb)

        # ---- per class-slot k ----
        pending_tail = None
        for k in range(K):
            # supports slice: 4 x [128, 2000] (1 MB contiguous DMAs),
            # k=0 prefetched before the preamble
            s_sb = s_tiles.pop(k) if k in s_tiles else load_s(k)

            # ks[d, nij] = Wqk @ s   (weight-stationary)
            ks_sb = kvpool.tile([128, NIJ], F32, tag="ks")
            for j in range(KSJ):
                ks_ps = ps_mm.tile([D, KSW], F32, tag="mm_ps")
                for cc in range(CC):
                    nc.tensor.matmul(ks_ps, _r(wq_sb[:, cc, :]),
                                     _r(s_sb[cc][:, j * KSW:(j + 1) * KSW]),
                                     start=(cc == 0), stop=(cc == CC - 1))
                nc.vector.tensor_copy(_r(ks_sb[:, j * KSW:(j + 1) * KSW]), ks_ps)

            # v[c, nij] = Wv @ s (weight-stationary), then PE-transpose
            # 128x125 blocks to get vT[nij, c] for the out-matmul weights
            v_sb = kvpool.tile([D, NIJ], F32, tag="v")
            for j in range(KSJ):
                v_ps = ps_mm.tile([D, KSW], F32, tag="mm_ps")
                for cc in range(CC):
                    nc.tensor.matmul(v_ps, _r(wv_sb[:, cc, :D]),
                                     _r(s_sb[cc][:, j * KSW:(j + 1) * KSW]),
                                     start=(cc == 0), stop=(cc == CC - 1))
                nc.vector.tensor_copy(_r(v_sb[:, j * KSW:(j + 1) * KSW]), v_ps)
            vt_sb = kvpool.tile([CH, NT, D], F32, tag="vt")
            for t in range(NT):
                vt_ps = ps_mm.tile([CH, DW], F32, tag="mm_ps")
                nc.tensor.matmul(_r(vt_ps[:, :D]), _r(v_sb[:, t * CH:(t + 1) * CH]),
                                 _r(identity), is_transpose=True,
                                 start=True, stop=True)
                if t % 2 == 0:
                    nc.vector.tensor_copy(_r(vt_sb[:, t, :]), vt_ps[:, :D])
                else:
                    nc.scalar.copy(_r(vt_sb[:, t, :]), vt_ps[:, :D])

            # emit previous k's normalization tail now: its bc-matmul lands
            # behind this k's projection matmuls in the in-order PE queue, so
            # the reciprocal chain resolves without stalling the PE
            if pending_tail is not None:
                pending_tail()
                pending_tail = None

            # attention chunks: simT -> exp -> accumulate out and expsum
            o_ps = ps_out.tile([D, HW], F32, tag="out_ps")
            sum_ps = ps_sum.tile([1, HW], F32, tag="sum_ps")
            LAG = 3   # run out/sum matmuls 3 chunks behind sim/exp so the
            e_tiles = {}  # PE queue never stalls on the exp of the same chunk
            for t in range(NT + LAG):
                if t < NT:
                    sim_ps = ps_sim.tile([CH, HW], F32, tag="sim_ps")
                    nc.tensor.matmul(sim_ps, _r(ks_sb[:, t * CH:(t + 1) * CH]),
                                     _r(q_sb), start=True, stop=True)
                    e_sb = epool.tile([CH, HW], F32, tag="e")
                    nc.scalar.activation(_r(e_sb), sim_ps,
                                         mybir.ActivationFunctionType.Exp,
                                         scale=SCALE)
                    e_tiles[t] = e_sb
                if t >= LAG:
                    td = t - LAG
                    e_sb = e_tiles[td]
                    nc.tensor.matmul(o_ps, _r(vt_sb[:, td, :]), _r(e_sb),
                                     start=(td == 0), stop=(td == NT - 1))
                    if td % 2 == 1:
                        # pre-add exp chunk pairs on the idle GpSimd engine so
                        # the partition-reduction needs 8 ones-matmuls, not 16
                        ep = epool.tile([CH, HW], F32, tag="ep")
                        with nc.allow_low_precision(reason="softmax denom"):
                            nc.gpsimd.tensor_add(_r(ep), _r(e_tiles.pop(td - 1)),
                                                 _r(e_tiles.pop(td)))
                        p = td // 2
                        nc.tensor.matmul(sum_ps, _r(ones_col), _r(ep),
                                         start=(p == 0), stop=(p == NT // 2 - 1))

            recip = epool.tile([1, HW], F32, tag="recip")
            nc.vector.reciprocal_approx_fast(recip, sum_ps)

            def make_tail(k=k, o_ps=o_ps, recip=recip):
                def tail():
                    # partition-broadcast via outer product, then normalize
                    bc_ps = ps_sim.tile([D, HW], F32, tag="sim_ps")
                    nc.tensor.matmul(bc_ps, ones_row, recip,
                                     start=True, stop=True)
                    bc_sb = epool.tile([D, HW], F32, tag="bc")
                    nc.vector.tensor_copy(bc_sb, bc_ps)
                    o_sb = opool.tile([D, HW], F32, tag="osb")
                    nc.vector.tensor_mul(o_sb, o_ps, bc_sb)
                    nc.scalar.dma_start(out=out_d[k], in_=o_sb)
                return tail
            pending_tail = make_tail()

        pending_tail()

    nc.compile()
    return nc


def _get_nc():
    if "nc" not in _CACHE:
        _CACHE["nc"] = _build_program()
    return _CACHE["nc"]


def _prep_weights(Wqk, Wv):
    wqkT = np.ascontiguousarray(np.asarray(Wqk, np.float32).T).reshape(CC, 128, D)
    wvT = np.ascontiguousarray(np.asarray(Wv, np.float32).T).reshape(CC, 128, D)
    # pad to DW cols (f32r full-rate needs moving dim >= 256; pad never read)
    wvT = np.ascontiguousarray(np.concatenate([wvT, wvT], axis=2))
    return wqkT, wvT


def _prep_core_inputs(query_repr, supports_repr, wqkT, wvT, b):
    xq = np.ascontiguousarray(query_repr[b].reshape(CC, 128, HW))
    # supports [K,N,C,H,W] -> [K, CC, 128, N*HW] with C chunked on partitions
    s = supports_repr[b].reshape(K, N, CC, 128, HW)
    s = np.ascontiguousarray(s.transpose(0, 2, 3, 1, 4)).reshape(K, CC, 128, NIJ)
    return {"xq": xq, "s": s, "wqkT": wqkT, "wvT": wvT}


def kernel(query_repr, supports_repr, Wqk, Wv):
    query_repr = np.asarray(query_repr, dtype=np.float32)
    supports_repr = np.asarray(supports_repr, dtype=np.float32)
    wqkT, wvT = _prep_weights(Wqk, Wv)

    nc = _get_nc()
    in_maps = [_prep_core_inputs(query_repr, supports_repr, wqkT, wvT, b)
               for b in range(B)]
    res = run_bass_kernel_spmd(nc, in_maps, list(range(B))).results

    query_v = np.stack([res[b]["qv"] for b in range(B)]).reshape(B, D, H, W)
    out = np.stack([res[b]["out"] for b in range(B)]).reshape(B, K, D, H, W)
    return query_v.astype(np.float32), out.astype(np.float32)


# revision 17
# speedup vs baseline: 2.3695x; 1.0634x over previous
"""Trainium2 Bass kernel for nn_CrossTransformer (cross-attention over support set).

Contract: kernel(**inputs) takes FULL inputs (query_repr [8,512,20,20],
supports_repr [8,5,5,512,20,20], Wqk [128,512], Wv [128,512]) and returns the
full outputs (query_v [8,128,20,20], out [8,5,128,20,20]) as float32, matching
reference():
    query_v = Wv @ query_repr                       (1x1 conv)
    sim     = (Wqk @ query)^T (Wqk @ supports) * 128^-0.5
    attn    = softmax over (n,i,j) of supports
    out     = attn @ (Wv @ supports)

Sharding: pure data-parallel over batch b -- each of the 8 NeuronCores handles
one batch element; no collectives.

Per-core strategy:
  - All hot matmuls run in float32r (single-pass fp32; 4x faster than fp32's
    LOW_HIGH two-pass mode at moving-dim >= 256; measured scale-rel err
    ~1.6e-4). Operand chains are f32r-typed end-to-end via bitcast views.
  - sim^T [nij, hw] = ks^T q with both operands in natural layout.
  - v^T for the out-matmul weights comes from weight-stationary v = Wv @ s
    followed by PE-mode transposes of 128x125 blocks (f32r transpose).
  - softmax over nij (the partition dim): exp on ScalarE, two sim chunks fused
    per ACTIVATE (the 352 ns per-op overhead is 56% of a single chunk); no
    max-subtraction needed (sim*scale ~ N(0,1), |max| ~ 7). Denominator: exp
    pairs pre-added on GpSimd, then 8 ones-matmuls accumulate the partition
    reduction in PSUM; 1/x via reciprocal_approx_fast; normalization by an
    outer-product broadcast matmul + GpSimd multiply (SBUF-only operands).
  - The in-order PE queue is kept fed: out/sum matmuls lag sim/exp by LAG
    pairs, and each k's normalization matmul is emitted after k+1's
    projection matmuls so dependency tails never stall the PE.
"""

import numpy as np
from contextlib import ExitStack

import concourse.bass as bass
import concourse.tile as tile
from concourse import bacc, masks, mybir
from concourse.bass_utils import run_bass_kernel_spmd

# Problem shape (hardcoded per harness contract)
B, K, N, C, D = 8, 5, 5, 512, 128
H = W = 20
HW = H * W                 # 400
NIJ = N * HW               # 2000
CC = C // 128              # 4 contraction chunks of 128
NT = 16                    # nij chunks
NP = NT // 2               # fused sim/exp pairs
CH = NIJ // NT             # 125 rows per chunk
KSJ = 4                    # ks/v free-dim chunks
KSW = NIJ // KSJ           # 500 (<= 512 f32 moving-operand limit)
SIMW = 1024                # fused sim psum tile: cols [0:400] + [512:912]
SCALE = float(D) ** -0.5
F32 = mybir.dt.float32
F32R = mybir.dt.float32r

_CACHE = {}


def _r(ap):
    """View an f32 AP as float32r for single-pass PE matmul."""
    return ap.bitcast(F32R)


def _build_program():
    nc = bacc.Bacc("TRN2", target_bir_lowering=False, debug=False)

    xq_d = nc.dram_tensor("xq", [CC, 128, HW], F32, kind="ExternalInput").ap()
    s_d = nc.dram_tensor("s", [K, CC, 128, NIJ], F32, kind="ExternalInput").ap()
    wq_d = nc.dram_tensor("wqkT", [CC, 128, D], F32, kind="ExternalInput").ap()
    wv_d = nc.dram_tensor("wvT", [CC, 128, D], F32, kind="ExternalInput").ap()
    qv_d = nc.dram_tensor("qv", [D, HW], F32, kind="ExternalOutput").ap()
    out_d = nc.dram_tensor("out", [K, D, HW], F32, kind="ExternalOutput").ap()

    with ExitStack() as ctx:
        tc = ctx.enter_context(tile.TileContext(nc))

        consts = ctx.enter_context(tc.tile_pool(name="consts", bufs=1))
        spool = ctx.enter_context(tc.tile_pool(name="spool", bufs=3))
        kvpool = ctx.enter_context(tc.tile_pool(name="kvpool", bufs=2))
        epool = ctx.enter_context(tc.tile_pool(name="epool", bufs=4))
        opool = ctx.enter_context(tc.tile_pool(name="opool", bufs=2))
        # PSUM budget (8 banks): mm 2 + fused sim 2x2 + out/sum shared 2
        ps_mm = ctx.enter_context(tc.tile_pool(name="ps_mm", bufs=2, space="PSUM"))
        ps_sim = ctx.enter_context(tc.tile_pool(name="ps_sim", bufs=2, space="PSUM"))
        ps_os = ctx.enter_context(tc.tile_pool(name="ps_os", bufs=2, space="PSUM"))

        def load_s(k, granular=False):
            tiles = [spool.tile([128, NIJ], F32, tag=f"s{cc}",
                                name=f"s_sb{k}_{cc}") for cc in range(CC)]
            if granular:
                # j-granular 256 KB DMAs so the first ks group starts early
                for j in range(KSJ):
                    for cc in range(CC):
                        nc.sync.dma_start(
                            out=_r(tiles[cc][:, j * KSW:(j + 1) * KSW]),
                            in_=_r(s_d[k, cc, :, j * KSW:(j + 1) * KSW]))
            else:
                for cc in range(CC):
                    nc.sync.dma_start(out=_r(tiles[cc]), in_=_r(s_d[k, cc]))
            return tiles

        # ---- small inputs on the scalar HWDGE ring (parallel to supports
        # loads on the sync ring), then the first supports slice
        wq_sb = consts.tile([128, CC, D], F32)
        wv_sb = consts.tile([128, CC, D], F32)
        xq_sb = consts.tile([128, CC, HW], F32)
        for cc in range(CC):
            nc.scalar.dma_start(out=_r(xq_sb[:, cc, :]), in_=_r(xq_d[cc]))
            nc.scalar.dma_start(out=_r(wq_sb[:, cc, :]), in_=_r(wq_d[cc]))
            nc.scalar.dma_start(out=_r(wv_sb[:, cc, :]), in_=_r(wv_d[cc]))
        s_tiles = {0: load_s(0, granular=True)}

        id_tmp = consts.tile([D, D], F32)
        masks.make_identity(nc, id_tmp)
        identity = consts.tile([D, D], F32)
        nc.vector.tensor_copy(_r(identity), id_tmp)
        ones_tmp = consts.tile([CH, 1], F32)
        nc.vector.memset(ones_tmp, 1.0)
        ones_col = consts.tile([CH, 1], F32)
        nc.vector.tensor_copy(_r(ones_col), ones_tmp)
        ones_row = consts.tile([1, D], F32)
        nc.vector.memset(ones_row, 1.0)

        # ---- q / qv projections (once per batch) ----
        q_ps = ps_os.tile([D, HW], F32, tag="os_ps")
        for cc in range(CC):
            nc.tensor.matmul(q_ps, _r(wq_sb[:, cc, :]), _r(xq_sb[:, cc, :]),
                             start=(cc == 0), stop=(cc == CC - 1))
        q_sb = consts.tile([D, HW], F32)
        nc.vector.tensor_copy(_r(q_sb), q_ps)

        qv_ps = ps_os.tile([D, HW], F32, tag="os_ps")
        for cc in range(CC):
            nc.tensor.matmul(qv_ps, _r(wv_sb[:, cc, :]), _r(xq_sb[:, cc, :]),
                             start=(cc == 0), stop=(cc == CC - 1))
        qv_sb = consts.tile([D, HW], F32)
        nc.vector.tensor_copy(qv_sb, qv_ps)
        nc.scalar.dma_start(out=qv_d, in_=qv_sb)

        # ---- per class-slot k ----
        pending_tail = None
        for k in range(K):
            s_sb = s_tiles.pop(k) if k in s_tiles else load_s(k)

            # ks[d, nij] = Wqk @ s   (weight-stationary)
            ks_sb = kvpool.tile([128, NIJ], F32, tag="ks")
            for j in range(KSJ):
                ks_ps = ps_mm.tile([D, KSW], F32, tag="mm_ps")
                for cc in range(CC):
                    nc.tensor.matmul(ks_ps, _r(wq_sb[:, cc, :]),
                                     _r(s_sb[cc][:, j * KSW:(j + 1) * KSW]),
                                     start=(cc == 0), stop=(cc == CC - 1))
                nc.vector.tensor_copy(_r(ks_sb[:, j * KSW:(j + 1) * KSW]), ks_ps)

            # v[c, nij] = Wv @ s (weight-stationary), then PE-transpose
            # 128x125 blocks to get vT[nij, c] for the out-matmul weights
            v_sb = kvpool.tile([D, NIJ], F32, tag="v")
            for j in range(KSJ):
                v_ps = ps_mm.tile([D, KSW], F32, tag="mm_ps")
                for cc in range(CC):
                    nc.tensor.matmul(v_ps, _r(wv_sb[:, cc, :]),
                                     _r(s_sb[cc][:, j * KSW:(j + 1) * KSW]),
                                     start=(cc == 0), stop=(cc == CC - 1))
                nc.vector.tensor_copy(_r(v_sb[:, j * KSW:(j + 1) * KSW]), v_ps)
            vt_sb = kvpool.tile([CH, NT, D], F32, tag="vt")
            for t in range(NT):
                vt_ps = ps_mm.tile([CH, D], F32, tag="mm_ps")
                nc.tensor.matmul(_r(vt_ps), _r(v_sb[:, t * CH:(t + 1) * CH]),
                                 _r(identity), is_transpose=True,
                                 start=True, stop=True)
                if t % 2 == 0:
                    nc.vector.tensor_copy(_r(vt_sb[:, t, :]), vt_ps)
                else:
                    nc.scalar.copy(_r(vt_sb[:, t, :]), vt_ps)

            # emit previous k's normalization tail now: its bc-matmul lands
            # behind this k's projection matmuls in the in-order PE queue, so
            # the reciprocal chain resolves without stalling the PE
            if pending_tail is not None:
                pending_tail()
                pending_tail = None

            # attention: fused sim pairs -> one exp per pair -> lagged
            # out/sum matmuls so the PE never waits on the same pair's exp
            o_ps = ps_os.tile([D, HW], F32, tag="os_ps")
            sum_ps = ps_os.tile([1, HW], F32, tag="os_ps")
            LAG = 2
            e_tiles = {}
            for pt in range(NP + LAG):
                if pt < NP:
                    sim_ps = ps_sim.tile([CH, SIMW], F32, tag="sim_ps")
                    for h in range(2):
                        t = 2 * pt + h
                        nc.tensor.matmul(
                            sim_ps[:, h * 512:h * 512 + HW],
                            _r(ks_sb[:, t * CH:(t + 1) * CH]), _r(q_sb),
                            start=True, stop=True)
                    e_sb = epool.tile([CH, 2, HW], F32, tag="e")
                    sim_view = sim_ps[:].rearrange(
                        "p (g x) -> p g x", g=2)[:, :, :HW]
                    nc.scalar.activation(_r(e_sb), sim_view,
                                         mybir.ActivationFunctionType.Exp,
                                         scale=SCALE)
                    e_tiles[pt] = e_sb
                if pt >= LAG:
                    pd = pt - LAG
                    e_sb = e_tiles.pop(pd)
                    for h in range(2):
                        td = 2 * pd + h
                        nc.tensor.matmul(o_ps, _r(vt_sb[:, td, :]),
                                         _r(e_sb[:, h, :]),
                                         start=(td == 0), stop=(td == NT - 1))
                    # pre-add the exp pair on GpSimd, then one ones-matmul
                    # accumulates the softmax denominator per pair
                    ep = epool.tile([CH, HW], F32, tag="ep")
                    with nc.allow_low_precision(reason="softmax denominator"):
                        nc.gpsimd.tensor_add(_r(ep), _r(e_sb[:, 0, :]),
                                             _r(e_sb[:, 1, :]))
                    nc.tensor.matmul(sum_ps, _r(ones_col), _r(ep),
                                     start=(pd == 0), stop=(pd == NP - 1))

            # evict o_ps/sum_ps promptly so k+1's attention gets the banks
            o_unorm = opool.tile([D, HW], F32, tag="o_unorm")
            nc.vector.tensor_copy(o_unorm, o_ps)
            recip = epool.tile([1, HW], F32, tag="recip")
            nc.vector.reciprocal_approx_fast(recip, sum_ps)

            def make_tail(k=k, o_unorm=o_unorm, recip=recip):
                def tail():
                    # partition-broadcast via outer product, then normalize
                    bc_ps = ps_mm.tile([D, HW], F32, tag="mm_ps")
                    nc.tensor.matmul(bc_ps, ones_row, recip,
                                     start=True, stop=True)
                    bc_sb = epool.tile([D, HW], F32, tag="bc")
                    nc.vector.tensor_copy(bc_sb, bc_ps)
                    o_sb = opool.tile([D, HW], F32, tag="osb")
                    nc.gpsimd.tensor_mul(o_sb, o_unorm, bc_sb)
                    nc.scalar.dma_start(out=out_d[k], in_=o_sb)
                return tail
            pending_tail = make_tail()

        pending_tail()

    nc.compile()
    return nc


def _get_nc():
    if "nc" not in _CACHE:
        _CACHE["nc"] = _build_program()
    return _CACHE["nc"]


def _prep_weights(Wqk, Wv):
    wqkT = np.ascontiguousarray(np.asarray(Wqk, np.float32).T).reshape(CC, 128, D)
    wvT = np.ascontiguousarray(np.asarray(Wv, np.float32).T).reshape(CC, 128, D)
    return wqkT, wvT


def _prep_core_inputs(query_repr, supports_repr, wqkT, wvT, b):
    xq = np.ascontiguousarray(query_repr[b].reshape(CC, 128, HW))
    # supports [K,N,C,H,W] -> [K, CC, 128, N*HW] with C chunked on partitions
    s = supports_repr[b].reshape(K, N, CC, 128, HW)
    s = np.ascontiguousarray(s.transpose(0, 2, 3, 1, 4)).reshape(K, CC, 128, NIJ)
    return {"xq": xq, "s": s, "wqkT": wqkT, "wvT": wvT}


def kernel(query_repr, supports_repr, Wqk, Wv):
    query_repr = np.asarray(query_repr, dtype=np.float32)
    supports_repr = np.asarray(supports_repr, dtype=np.float32)
    wqkT, wvT = _prep_weights(Wqk, Wv)

    nc = _get_nc()
    in_maps = [_prep_core_inputs(query_repr, supports_repr, wqkT, wvT, b)
               for b in range(B)]
    res = run_bass_kernel_spmd(nc, in_maps, list(range(B))).results

    query_v = np.stack([res[b]["qv"] for b in range(B)]).reshape(B, D, H, W)
    out = np.stack([res[b]["out"] for b in range(B)]).reshape(B, K, D, H, W)
    return query_v.astype(np.float32), out.astype(np.float32)


# revision 18
# speedup vs baseline: 2.4657x; 1.0406x over previous
"""Trainium2 Bass kernel for nn_CrossTransformer (cross-attention over support set).

Contract: kernel(**inputs) takes FULL inputs (query_repr [8,512,20,20],
supports_repr [8,5,5,512,20,20], Wqk [128,512], Wv [128,512]) and returns the
full outputs (query_v [8,128,20,20], out [8,5,128,20,20]) as float32, matching
reference():
    query_v = Wv @ query_repr                       (1x1 conv)
    sim     = (Wqk @ query)^T (Wqk @ supports) * 128^-0.5
    attn    = softmax over (n,i,j) of supports
    out     = attn @ (Wv @ supports)

Sharding: pure data-parallel over batch b -- each of the 8 NeuronCores handles
one batch element; no collectives.

Per-core strategy:
  - All hot matmuls run in float32r (single-pass fp32; 4x faster than fp32's
    LOW_HIGH two-pass mode at moving-dim >= 256; measured scale-rel err
    ~1.6e-4). Operand chains are f32r-typed end-to-end via bitcast views.
  - sim^T [nij, hw] = ks^T q with both operands in natural layout.
  - v^T for the out-matmul weights comes from weight-stationary v = Wv @ s
    followed by PE-mode transposes of 128x125 blocks (f32r transpose).
  - softmax over nij (the partition dim): exp on ScalarE, two sim chunks fused
    per ACTIVATE (the 352 ns per-op overhead is 56% of a single chunk); no
    max-subtraction needed (sim*scale ~ N(0,1), |max| ~ 7). Denominator: exp
    pairs pre-added on GpSimd, then 8 ones-matmuls accumulate the partition
    reduction in PSUM; 1/x via reciprocal_approx_fast; normalization by an
    outer-product broadcast matmul + GpSimd multiply (SBUF-only operands).
  - The in-order PE queue is kept fed: out/sum matmuls lag sim/exp by LAG
    pairs, and each k's normalization matmul is emitted after k+1's
    projection matmuls so dependency tails never stall the PE.
"""

import numpy as np
from contextlib import ExitStack

import concourse.bass as bass
import concourse.tile as tile
from concourse import bacc, masks, mybir
from concourse.bass_utils import run_bass_kernel_spmd

# Problem shape (hardcoded per harness contract)
B, K, N, C, D = 8, 5, 5, 512, 128
H = W = 20
HW = H * W                 # 400
NIJ = N * HW               # 2000
CC = C // 128              # 4 contraction chunks of 128
NT = 16                    # nij chunks
NP = NT // 2               # fused sim/exp pairs
CH = NIJ // NT             # 125 rows per chunk
KSJ = 4                    # ks/v free-dim chunks
KSW = NIJ // KSJ           # 500 (<= 512 f32 moving-operand limit)
SIMW = 1024                # fused sim psum tile: cols [0:400] + [512:912]
SCALE = float(D) ** -0.5
F32 = mybir.dt.float32
F32R = mybir.dt.float32r

_CACHE = {}


def _r(ap):
    """View an f32 AP as float32r for single-pass PE matmul."""
    return ap.bitcast(F32R)


def _build_program():
    nc = bacc.Bacc("TRN2", target_bir_lowering=False, debug=False)

    xq_d = nc.dram_tensor("xq", [CC, 128, HW], F32, kind="ExternalInput").ap()
    s_d = nc.dram_tensor("s", [K, CC, 128, NIJ], F32, kind="ExternalInput").ap()
    wq_d = nc.dram_tensor("wqkT", [CC, 128, D], F32, kind="ExternalInput").ap()
    wv_d = nc.dram_tensor("wvT", [CC, 128, D], F32, kind="ExternalInput").ap()
    qv_d = nc.dram_tensor("qv", [D, HW], F32, kind="ExternalOutput").ap()
    out_d = nc.dram_tensor("out", [K, D, HW], F32, kind="ExternalOutput").ap()

    with ExitStack() as ctx:
        tc = ctx.enter_context(tile.TileContext(nc))

        consts = ctx.enter_context(tc.tile_pool(name="consts", bufs=1))
        spool = ctx.enter_context(tc.tile_pool(name="spool", bufs=3))
        kvpool = ctx.enter_context(tc.tile_pool(name="kvpool", bufs=2))
        epool = ctx.enter_context(tc.tile_pool(name="epool", bufs=4))
        opool = ctx.enter_context(tc.tile_pool(name="opool", bufs=2))
        # PSUM budget (8 banks): mm 2 + fused sim 2x2 + out/sum shared 2
        ps_mm = ctx.enter_context(tc.tile_pool(name="ps_mm", bufs=2, space="PSUM"))
        ps_sim = ctx.enter_context(tc.tile_pool(name="ps_sim", bufs=2, space="PSUM"))
        ps_os = ctx.enter_context(tc.tile_pool(name="ps_os", bufs=2, space="PSUM"))

        def load_s(k, granular=False):
            tiles = [spool.tile([128, NIJ], F32, tag=f"s{cc}",
                                name=f"s_sb{k}_{cc}") for cc in range(CC)]
            if granular:
                # j-granular 256 KB DMAs so the first ks group starts early
                for j in range(KSJ):
                    for cc in range(CC):
                        nc.sync.dma_start(
                            out=_r(tiles[cc][:, j * KSW:(j + 1) * KSW]),
                            in_=_r(s_d[k, cc, :, j * KSW:(j + 1) * KSW]))
            else:
                for cc in range(CC):
                    nc.sync.dma_start(out=_r(tiles[cc]), in_=_r(s_d[k, cc]))
            return tiles

        # ---- small inputs on the scalar HWDGE ring (parallel to supports
        # loads on the sync ring), then the first supports slice
        wq_sb = consts.tile([128, CC, D], F32)
        wv_sb = consts.tile([128, CC, D], F32)
        xq_sb = consts.tile([128, CC, HW], F32)
        for cc in range(CC):
            nc.scalar.dma_start(out=_r(xq_sb[:, cc, :]), in_=_r(xq_d[cc]))
            nc.scalar.dma_start(out=_r(wq_sb[:, cc, :]), in_=_r(wq_d[cc]))
            nc.scalar.dma_start(out=_r(wv_sb[:, cc, :]), in_=_r(wv_d[cc]))
        s_tiles = {0: load_s(0, granular=True)}

        id_tmp = consts.tile([D, D], F32)
        masks.make_identity(nc, id_tmp)
        identity = consts.tile([D, D], F32)
        nc.vector.tensor_copy(_r(identity), id_tmp)
        ones_tmp = consts.tile([CH, 1], F32)
        nc.vector.memset(ones_tmp, 1.0)
        ones_col = consts.tile([CH, 1], F32)
        nc.vector.tensor_copy(_r(ones_col), ones_tmp)
        ones_row = consts.tile([1, D], F32)
        nc.vector.memset(ones_row, 1.0)
        ones_row_r = consts.tile([1, D], F32)
        nc.vector.tensor_copy(_r(ones_row_r), ones_row)

        # ---- q / qv projections (once per batch) ----
        q_ps = ps_os.tile([D, HW], F32, tag="os_ps")
        for cc in range(CC):
            nc.tensor.matmul(q_ps, _r(wq_sb[:, cc, :]), _r(xq_sb[:, cc, :]),
                             start=(cc == 0), stop=(cc == CC - 1))
        q_sb = consts.tile([D, HW], F32)
        nc.vector.tensor_copy(_r(q_sb), q_ps)

        qv_ps = ps_os.tile([D, HW], F32, tag="os_ps")
        for cc in range(CC):
            nc.tensor.matmul(qv_ps, _r(wv_sb[:, cc, :]), _r(xq_sb[:, cc, :]),
                             start=(cc == 0), stop=(cc == CC - 1))
        qv_sb = consts.tile([D, HW], F32)
        nc.vector.tensor_copy(qv_sb, qv_ps)
        nc.scalar.dma_start(out=qv_d, in_=qv_sb)

        # ---- per class-slot k ----
        pending_tail = None
        for k in range(K):
            s_sb = s_tiles.pop(k) if k in s_tiles else load_s(k)

            # ks[d, nij] = Wqk @ s   (weight-stationary)
            ks_sb = kvpool.tile([128, NIJ], F32, tag="ks")
            for j in range(KSJ):
                ks_ps = ps_mm.tile([D, KSW], F32, tag="mm_ps")
                for cc in range(CC):
                    nc.tensor.matmul(ks_ps, _r(wq_sb[:, cc, :]),
                                     _r(s_sb[cc][:, j * KSW:(j + 1) * KSW]),
                                     start=(cc == 0), stop=(cc == CC - 1))
                nc.vector.tensor_copy(_r(ks_sb[:, j * KSW:(j + 1) * KSW]), ks_ps)

            # v[c, nij] = Wv @ s (weight-stationary), then PE-transpose
            # 128x125 blocks to get vT[nij, c] for the out-matmul weights
            v_sb = kvpool.tile([D, NIJ], F32, tag="v")
            for j in range(KSJ):
                v_ps = ps_mm.tile([D, KSW], F32, tag="mm_ps")
                for cc in range(CC):
                    nc.tensor.matmul(v_ps, _r(wv_sb[:, cc, :]),
                                     _r(s_sb[cc][:, j * KSW:(j + 1) * KSW]),
                                     start=(cc == 0), stop=(cc == CC - 1))
                nc.vector.tensor_copy(_r(v_sb[:, j * KSW:(j + 1) * KSW]), v_ps)
            vt_sb = kvpool.tile([CH, NT, D], F32, tag="vt")
            for t in range(NT):
                vt_ps = ps_mm.tile([CH, D], F32, tag="mm_ps")
                nc.tensor.matmul(_r(vt_ps), _r(v_sb[:, t * CH:(t + 1) * CH]),
                                 _r(identity), is_transpose=True,
                                 start=True, stop=True)
                if t % 2 == 0:
                    nc.vector.tensor_copy(_r(vt_sb[:, t, :]), vt_ps)
                else:
                    nc.scalar.copy(_r(vt_sb[:, t, :]), vt_ps)

            # emit previous k's normalization tail now: its bc-matmul lands
            # behind this k's projection matmuls in the in-order PE queue, so
            # the reciprocal chain resolves without stalling the PE
            if pending_tail is not None:
                pending_tail()
                pending_tail = None

            # attention: fused sim pairs -> one exp per pair -> lagged
            # out/sum matmuls so the PE never waits on the same pair's exp
            o_ps = ps_os.tile([D, HW], F32, tag="os_ps")
            sum_ps = ps_os.tile([1, HW], F32, tag="os_ps")
            LAG = 2
            e_tiles = {}
            for pt in range(NP + LAG):
                if pt < NP:
                    sim_ps = ps_sim.tile([CH, SIMW], F32, tag="sim_ps")
                    for h in range(2):
                        t = 2 * pt + h
                        nc.tensor.matmul(
                            sim_ps[:, h * 512:h * 512 + HW],
                            _r(ks_sb[:, t * CH:(t + 1) * CH]), _r(q_sb),
                            start=True, stop=True)
                    e_sb = epool.tile([CH, 2, HW], F32, tag="e")
                    sim_view = sim_ps[:].rearrange(
                        "p (g x) -> p g x", g=2)[:, :, :HW]
                    nc.scalar.activation(_r(e_sb), sim_view,
                                         mybir.ActivationFunctionType.Exp,
                                         scale=SCALE)
                    e_tiles[pt] = e_sb
                if pt >= LAG:
                    pd = pt - LAG
                    e_sb = e_tiles.pop(pd)
                    for h in range(2):
                        td = 2 * pd + h
                        nc.tensor.matmul(o_ps, _r(vt_sb[:, td, :]),
                                         _r(e_sb[:, h, :]),
                                         start=(td == 0), stop=(td == NT - 1))
                    # pre-add the exp pair on GpSimd, then one ones-matmul
                    # accumulates the softmax denominator per pair
                    ep = epool.tile([CH, HW], F32, tag="ep")
                    with nc.allow_low_precision(reason="softmax denominator"):
                        nc.gpsimd.tensor_add(_r(ep), _r(e_sb[:, 0, :]),
                                             _r(e_sb[:, 1, :]))
                    nc.tensor.matmul(sum_ps, _r(ones_col), _r(ep),
                                     start=(pd == 0), stop=(pd == NP - 1))

            # reciprocal first so the bc matmul unblocks ASAP, then evict
            # o_ps/sum_ps so k+1's attention gets the banks
            recip = epool.tile([1, HW], F32, tag="recip")
            nc.vector.reciprocal_approx_fast(recip, sum_ps)
            recip_r = epool.tile([1, HW], F32, tag="recip_r")
            nc.vector.tensor_copy(_r(recip_r), recip)
            last = k == K - 1
            if not last:
                o_unorm = opool.tile([D, HW], F32, tag="o_unorm")
                nc.vector.tensor_copy(o_unorm, o_ps)
            else:
                o_unorm = o_ps

            def make_tail(k=k, o_unorm=o_unorm, recip_r=recip_r, last=last):
                def tail():
                    # partition-broadcast via outer product, then normalize
                    bc_ps = ps_mm.tile([D, HW], F32, tag="mm_ps")
                    nc.tensor.matmul(bc_ps, _r(ones_row_r), _r(recip_r),
                                     start=True, stop=True)
                    bc_sb = epool.tile([D, HW], F32, tag="bc")
                    nc.vector.tensor_copy(bc_sb, bc_ps)
                    o_sb = opool.tile([D, HW], F32, tag="osb")
                    if last:
                        nc.vector.tensor_mul(o_sb, o_unorm, bc_sb)
                    else:
                        nc.gpsimd.tensor_mul(o_sb, o_unorm, bc_sb)
                    nc.scalar.dma_start(out=out_d[k], in_=o_sb)
                return tail
            pending_tail = make_tail()

        pending_tail()

    nc.compile()
    return nc


def _get_nc():
    if "nc" not in _CACHE:
        _CACHE["nc"] = _build_program()
    return _CACHE["nc"]


def _prep_weights(Wqk, Wv):
    wqkT = np.ascontiguousarray(np.asarray(Wqk, np.float32).T).reshape(CC, 128, D)
    wvT = np.ascontiguousarray(np.asarray(Wv, np.float32).T).reshape(CC, 128, D)
    return wqkT, wvT


def _prep_core_inputs(query_repr, supports_repr, wqkT, wvT, b):
    xq = np.ascontiguousarray(query_repr[b].reshape(CC, 128, HW))
    # supports [K,N,C,H,W] -> [K, CC, 128, N*HW] with C chunked on partitions
    s = supports_repr[b].reshape(K, N, CC, 128, HW)
    s = np.ascontiguousarray(s.transpose(0, 2, 3, 1, 4)).reshape(K, CC, 128, NIJ)
    return {"xq": xq, "s": s, "wqkT": wqkT, "wvT": wvT}


def kernel(query_repr, supports_repr, Wqk, Wv):
    query_repr = np.asarray(query_repr, dtype=np.float32)
    supports_repr = np.asarray(supports_repr, dtype=np.float32)
    wqkT, wvT = _prep_weights(Wqk, Wv)

    nc = _get_nc()
    in_maps = [_prep_core_inputs(query_repr, supports_repr, wqkT, wvT, b)
               for b in range(B)]
    res = run_bass_kernel_spmd(nc, in_maps, list(range(B))).results

    query_v = np.stack([res[b]["qv"] for b in range(B)]).reshape(B, D, H, W)
    out = np.stack([res[b]["out"] for b in range(B)]).reshape(B, K, D, H, W)
    return query_v.astype(np.float32), out.astype(np.float32)
